# revision 12
# baseline (speedup 1.0000x reference)
"""Batch-OMP DictionaryLearning kernel for 8x Trainium2 NeuronCores.

Per core (data-parallel over batch, BLOC=2048 items = 16 tiles of 128):
  G = D @ D.T + eps*I, augmented with its diagonal as a 513th column and
  stored row-major in DRAM for per-item row gathers.
  hbar = Y @ D.T via PE (fp32).
  Incremental Cholesky-OMP (c-form), K=8 iterations:
    argmax |h| via h^2 + InstMax/InstMaxIndex (first-match tie semantics,
    matching jnp.argmax), indirect-DMA gather of the selected G_aug row,
    w_j = c_j[idx] via onehot multiply-reduce, unnormalized
    c'_k = GD - sum_j (w_j*rc_j) c'_j, rank-1 update h -= (yk_k*rc_k) c'_k.
  Batched back-substitution solves L^T x = yk; X assembled on PE via
  diag(x_k) @ onehot_k matmuls; Y_pred = X @ D via PE transposes.

Tiles processed in 3 passes of [6,6,4] (SBUF capacity for c' planes).
"""
import numpy as np

B, M, N, K = 16384, 256, 512, 8
NCORES = 8
BLOC = B // NCORES           # 2048
NT = BLOC // 128             # 16
PASS_T = [6, 6, 4]
TMAX = 6
DIAG_EPS = 1e-5
ROWW = 516                   # padded G_aug row width (513 used)
P = 128
import os as _os
PE_K_MIN = int(_os.environ.get("PE_K_MIN", "3"))
POOL_TILES = int(_os.environ.get("POOL_TILES", "0"))
TINY_GROUPS = int(_os.environ.get("TINY_GROUPS", "6"))
HSQ_DVE = int(_os.environ.get("HSQ_DVE", "0"))
PHG_POOL = int(_os.environ.get("PHG_POOL", "0"))
POOL_JMOD = int(_os.environ.get("POOL_JMOD", "0"))

_CACHE = {}


def _build_nc():
    import concourse.bacc as bacc
    import concourse.mybir as mybir
    from concourse.alu_op_type import AluOpType as aop
    from concourse.bass import IndirectOffsetOnAxis
    from concourse.tile import TileContext

    F32 = mybir.dt.float32
    U32 = mybir.dt.uint32
    AF = mybir.ActivationFunctionType
    AX = mybir.AxisListType.X

    nc = bacc.Bacc(None, target_bir_lowering=False)

    Yd = nc.dram_tensor("Y", [BLOC, M], F32, kind="ExternalInput")
    Dd = nc.dram_tensor("D", [N, M], F32, kind="ExternalInput")
    IOTAd = nc.dram_tensor("IOTA", [P, N], F32, kind="ExternalInput")
    M1d = nc.dram_tensor("M1", [P, 896], F32, kind="ExternalInput")
    XOUT = nc.dram_tensor("XOUT", [BLOC, N], F32, kind="ExternalOutput")
    YPOUT = nc.dram_tensor("YPOUT", [BLOC, M], F32, kind="ExternalOutput")

    with TileContext(nc) as tc:
        with (
            tc.tile_pool(name="dram", bufs=1, space="DRAM") as dpool,
            tc.tile_pool(name="const", bufs=1) as cpool,
            tc.tile_pool(name="mats", bufs=1) as mpool,
            tc.tile_pool(name="state", bufs=1) as spool,
            tc.tile_pool(name="work", bufs=2) as wpool,
            tc.tile_pool(name="ps_t", bufs=2, space="PSUM") as pst,
            tc.tile_pool(name="ps_mm", bufs=2, space="PSUM") as psm,
        ):
            GAUG = dpool.tile([N, ROWW], F32)
            HBARD = dpool.tile([NT, P, N], F32)

            iota = cpool.tile([P, N], F32)
            m1 = cpool.tile([P, 896], F32)
            nc.sync.dma_start(iota[:], IOTAd[:])
            nc.sync.dma_start(m1[:], M1d[:])
            IDM = m1[:, 384:512]                     # [128,128] identity
            IDM_full = cpool.tile([P, P], F32)
            nc.vector.tensor_copy(IDM_full[:], IDM)

            # ---------------- phase A: G_aug + hbar ----------------
            D_sb = mpool.tile([P, 4, M], F32)        # D rows chunked
            nc.sync.dma_start(D_sb[:], Dd.rearrange("(c p) m -> p c m", p=P))
            Dt = mpool.tile([P, 2, N], F32)          # D^T chunked over m
            for nch in range(4):
                for mc in range(2):
                    tp = pst.tile([P, P], F32, tag="tp")
                    nc.tensor.transpose(
                        tp[:], D_sb[:, nch, mc * P:(mc + 1) * P], IDM)
                    nc.scalar.copy(Dt[:, mc, nch * P:(nch + 1) * P], tp[:])

            G_sb = mpool.tile([P, 4, N], F32)
            diagS = mpool.tile([P, 4], F32)
            junkG = wpool.tile([P, N], F32, tag="junkG")
            for c in range(4):
                gp = psm.tile([P, N], F32, tag="mm")
                for mc in range(2):
                    nc.tensor.matmul(
                        gp[:], Dt[:, mc, c * P:(c + 1) * P], Dt[:, mc, :],
                        start=(mc == 0), stop=(mc == 1))
                m1v = m1[:, 384 - c * P: 896 - c * P]
                nc.vector.scalar_tensor_tensor(
                    G_sb[:, c, :], m1v, DIAG_EPS, gp[:], aop.mult, aop.add)
                nc.vector.scalar_tensor_tensor(
                    junkG[:], G_sb[:, c, :], 1.0, m1v, aop.bypass, aop.mult,
                    accum_out=diagS[:, c:c + 1])
            gaug_v = GAUG.rearrange("(c p) w -> p c w", p=P)
            for c in range(4):
                nc.sync.dma_start(gaug_v[:, c, 0:N], G_sb[:, c, :])
                nc.sync.dma_start(gaug_v[:, c, N:N + 1], diagS[:, c:c + 1])

            Yv = Yd.rearrange("(g p) m -> g p m", p=P)
            for g in range(NT):
                yt = wpool.tile([P, M], F32, tag="yt")
                nc.sync.dma_start(yt[:], Yv[g])
                yT = wpool.tile([P, 2, P], F32, tag="yT")
                for mc in range(2):
                    tp = pst.tile([P, P], F32, tag="tp")
                    nc.tensor.transpose(tp[:], yt[:, mc * P:(mc + 1) * P], IDM)
                    nc.scalar.copy(yT[:, mc, :], tp[:])
                hp = psm.tile([P, N], F32, tag="mm")
                for mc in range(2):
                    nc.tensor.matmul(hp[:], yT[:, mc, :], Dt[:, mc, :],
                                     start=(mc == 0), stop=(mc == 1))
                hb = wpool.tile([P, N], F32, tag="hb")
                nc.scalar.copy(hb[:], hp[:])
                nc.sync.dma_start(HBARD[g], hb[:])

            # ---------------- OMP state ----------------
            h_sb = spool.tile([P, TMAX, N], F32)
            cp_sb = spool.tile([P, TMAX, K, N], F32)
            GD = spool.tile([P, TMAX, ROWW], F32)
            hsq = spool.tile([P, 2, N], F32)
            onehot = spool.tile([P, 2, N], F32)
            mx8 = spool.tile([P, TMAX, 8], F32)
            ix8 = spool.tile([P, TMAX, 8], U32)
            ixu = spool.tile([P, NT, K], U32)
            xsave = spool.tile([P, NT, K], F32)
            hsel = spool.tile([P, TMAX], F32)
            idx1f = spool.tile([P, TMAX], F32)
            wraw = spool.tile([P, TMAX, K], F32)
            wnrm = spool.tile([P, TMAX, K], F32)
            wsq3 = spool.tile([P, TMAX, K], F32)
            wsq = spool.tile([P, TMAX], F32)
            wc2 = spool.tile([P, TMAX], F32)
            wc = spool.tile([P, TMAX], F32)
            rcS = spool.tile([P, TMAX, K], F32)
            ykS = spool.tile([P, TMAX, K], F32)
            gS = spool.tile([P, TMAX, K], F32)
            hscal = spool.tile([P, TMAX], F32)
            LS = spool.tile([P, TMAX, K, K], F32)
            junk = spool.tile([P, N], F32)
            junk2 = spool.tile([P, N], F32)
            junk3 = spool.tile([P, N], F32)
            xacc = spool.tile([P, TMAX], F32)
            tmpT = spool.tile([P, TMAX], F32)

            XOUTv = XOUT.rearrange("(g p) n -> g p n", p=P)
            YPOUTv = YPOUT.rearrange("(g p) m -> g p m", p=P)

            g0 = 0
            for pi, T in enumerate(PASS_T):
                for t in range(T):
                    nc.sync.dma_start(h_sb[:, t, :], HBARD[g0 + t])
                for k in range(K):
                    ngr = max(1, min(TINY_GROUPS, T))
                    szs = [T // ngr + (1 if i < T % ngr else 0)
                           for i in range(ngr)]
                    gb0 = 0
                    for sz in szs:
                        ga, gb = gb0, gb0 + sz
                        gb0 = gb
                        # ---- stage A ----
                        for t in range(ga, gb):
                            g = g0 + t
                            hq = hsq[:, t % 2, :]
                            oh = onehot[:, t % 2, :]
                            if HSQ_DVE:
                                nc.vector.tensor_mul(hq, h_sb[:, t, :],
                                                     h_sb[:, t, :])
                            else:
                                nc.scalar.activation(hq, h_sb[:, t, :],
                                                     AF.Square)
                            nc.vector.max(mx8[:, t, :], hq)
                            nc.vector.max_index(ix8[:, t, :], mx8[:, t, :], hq)
                            nc.vector.tensor_copy(ixu[:, g, k:k + 1],
                                                  ix8[:, t, 0:1])
                            nc.gpsimd.indirect_dma_start(
                                out=GD[:, t, :], out_offset=None,
                                in_=GAUG[:, :],
                                in_offset=IndirectOffsetOnAxis(
                                    ap=ixu[:, g, k:k + 1], axis=0))
                            nc.vector.tensor_copy(idx1f[:, t:t + 1],
                                                  ix8[:, t, 0:1])
                            nc.vector.tensor_scalar(
                                oh, iota[:], idx1f[:, t:t + 1], None,
                                aop.is_equal)
                            seng = nc.gpsimd if t < POOL_TILES else nc.vector
                            seng.scalar_tensor_tensor(
                                junk[:], oh, 1.0, h_sb[:, t, :],
                                aop.bypass, aop.mult,
                                accum_out=hsel[:, t:t + 1])
                            for j in range(k):
                                jeng = (nc.gpsimd if (POOL_JMOD and
                                        j % POOL_JMOD == POOL_JMOD - 1)
                                        else seng)
                                jeng.scalar_tensor_tensor(
                                    junk2[:] if jeng is nc.vector else junk3[:],
                                    cp_sb[:, t, j, :], 1.0, oh,
                                    aop.bypass, aop.mult,
                                    accum_out=wraw[:, t, j:j + 1])
                        # ---- batched tiny algebra (group slice) ----
                        if k > 0:
                            nc.vector.tensor_mul(
                                wnrm[:, ga:gb, 0:k], wraw[:, ga:gb, 0:k],
                                rcS[:, ga:gb, 0:k])
                            nc.vector.tensor_mul(
                                wsq3[:, ga:gb, 0:k], wnrm[:, ga:gb, 0:k],
                                wnrm[:, ga:gb, 0:k])
                            nc.vector.tensor_reduce(
                                wsq[:, ga:gb], wsq3[:, ga:gb, 0:k], AX,
                                aop.add)
                            nc.vector.tensor_sub(
                                wc2[:, ga:gb], GD[:, ga:gb, N], wsq[:, ga:gb])
                        else:
                            nc.vector.tensor_copy(wc2[:, ga:gb],
                                                  GD[:, ga:gb, N])
                        nc.vector.tensor_scalar_max(
                            wc2[:, ga:gb], wc2[:, ga:gb], DIAG_EPS)
                        nc.scalar.activation(wc[:, ga:gb], wc2[:, ga:gb],
                                             AF.Sqrt)
                        nc.vector.reciprocal(rcS[:, ga:gb, k], wc[:, ga:gb])
                        nc.vector.tensor_mul(
                            ykS[:, ga:gb, k], hsel[:, ga:gb],
                            rcS[:, ga:gb, k])
                        nc.vector.tensor_mul(
                            hscal[:, ga:gb], ykS[:, ga:gb, k],
                            rcS[:, ga:gb, k])
                        nc.vector.tensor_scalar_mul(
                            hscal[:, ga:gb], hscal[:, ga:gb], -1.0)
                        if k > 0:
                            nc.vector.tensor_mul(
                                gS[:, ga:gb, 0:k], wnrm[:, ga:gb, 0:k],
                                rcS[:, ga:gb, 0:k])
                            nc.vector.tensor_scalar_mul(
                                gS[:, ga:gb, 0:k], gS[:, ga:gb, 0:k], -1.0)
                            nc.vector.tensor_copy(
                                LS[:, ga:gb, k, 0:k], wnrm[:, ga:gb, 0:k])
                        nc.vector.tensor_copy(LS[:, ga:gb, k, k],
                                              wc[:, ga:gb])
                        # ---- stage C ----
                        for t in range(ga, gb):
                            if k == 0:
                                nc.scalar.copy(cp_sb[:, t, 0, :],
                                               GD[:, t, 0:N])
                            elif k >= PE_K_MIN:
                                cps = psm.tile([P, N], F32, tag="mm")
                                nc.tensor.matmul(cps[:], IDM_full[:],
                                                 GD[:, t, 0:N],
                                                 start=True, stop=False)
                                for j in range(k):
                                    dgj = wpool.tile([P, P], F32, tag="dgj")
                                    nc.scalar.activation(
                                        dgj[:], IDM, AF.Copy,
                                        scale=gS[:, t, j:j + 1])
                                    nc.tensor.matmul(cps[:], dgj[:],
                                                     cp_sb[:, t, j, :],
                                                     start=False,
                                                     stop=(j == k - 1))
                                nc.scalar.copy(cp_sb[:, t, k, :], cps[:])
                            else:
                                nc.vector.scalar_tensor_tensor(
                                    cp_sb[:, t, k, :], cp_sb[:, t, 0, :],
                                    gS[:, t, 0:1], GD[:, t, 0:N],
                                    aop.mult, aop.add)
                                for j in range(1, k):
                                    nc.vector.scalar_tensor_tensor(
                                        cp_sb[:, t, k, :], cp_sb[:, t, j, :],
                                        gS[:, t, j:j + 1], cp_sb[:, t, k, :],
                                        aop.mult, aop.add)
                            nc.vector.scalar_tensor_tensor(
                                h_sb[:, t, :], cp_sb[:, t, k, :],
                                hscal[:, t:t + 1], h_sb[:, t, :],
                                aop.mult, aop.add)
                # ---- back-substitution: L^T x = yk ----
                for i in range(K - 1, -1, -1):
                    nc.vector.tensor_copy(xacc[:, 0:T], ykS[:, 0:T, i])
                    for j in range(i + 1, K):
                        nc.vector.tensor_mul(
                            tmpT[:, 0:T], LS[:, 0:T, j, i],
                            xsave[:, g0:g0 + T, j])
                        nc.vector.tensor_sub(
                            xacc[:, 0:T], xacc[:, 0:T], tmpT[:, 0:T])
                    nc.vector.tensor_mul(
                        xsave[:, g0:g0 + T, i], xacc[:, 0:T], rcS[:, 0:T, i])
                # ---- phase G for this pass: X + Y_pred ----
                for t in range(T):
                    g = g0 + t
                    idxf = wpool.tile([P, K], F32, tag="idxf")
                    nc.vector.tensor_copy(idxf[:], ixu[:, g, :])
                    Xps = psm.tile([P, N], F32, tag="mm")
                    for k in range(K):
                        ohg = wpool.tile([P, N], F32, tag="ohg")
                        oeng = nc.gpsimd if PHG_POOL else nc.vector
                        oeng.tensor_scalar(
                            ohg[:], iota[:], idxf[:, k:k + 1], None,
                            aop.is_equal)
                        dgx = wpool.tile([P, P], F32, tag="dgx")
                        nc.scalar.activation(
                            dgx[:], IDM, AF.Copy, scale=xsave[:, g, k:k + 1])
                        nc.tensor.matmul(Xps[:], dgx[:], ohg[:],
                                         start=(k == 0), stop=(k == K - 1))
                    Xsb = wpool.tile([P, N], F32, tag="Xsb")
                    nc.scalar.copy(Xsb[:], Xps[:])
                    nc.sync.dma_start(XOUTv[g], Xsb[:])
                    Yps = psm.tile([P, M], F32, tag="mm")
                    for c in range(4):
                        tp = pst.tile([P, P], F32, tag="tp")
                        nc.tensor.transpose(
                            tp[:], Xsb[:, c * P:(c + 1) * P], IDM)
                        xT = wpool.tile([P, P], F32, tag="xT")
                        nc.scalar.copy(xT[:], tp[:])
                        nc.tensor.matmul(Yps[:], xT[:], D_sb[:, c, :],
                                         start=(c == 0), stop=(c == 3))
                    Ysb = wpool.tile([P, M], F32, tag="Ysb")
                    nc.scalar.copy(Ysb[:], Yps[:])
                    nc.sync.dma_start(YPOUTv[g], Ysb[:])
                g0 += T
    nc.compile()
    return nc


def _get_nc():
    if "nc" not in _CACHE:
        _CACHE["nc"] = _build_nc()
    return _CACHE["nc"]


def _host_consts():
    iota = np.tile(np.arange(N, dtype=np.float32), (P, 1))
    m1 = np.zeros((P, 896), np.float32)
    m1[np.arange(P), 384 + np.arange(P)] = 1.0
    return iota, m1


def kernel(Y, D, _trace=False):
    from concourse.bass_utils import run_bass_kernel_spmd

    Y = np.ascontiguousarray(Y, dtype=np.float32)
    D = np.ascontiguousarray(D, dtype=np.float32)
    nc = _get_nc()
    iota, m1 = _host_consts()
    in_maps = []
    for c in range(NCORES):
        in_maps.append({
            "Y": Y[c * BLOC:(c + 1) * BLOC],
            "D": D,
            "IOTA": iota,
            "M1": m1,
        })
    res = run_bass_kernel_spmd(nc, in_maps, list(range(NCORES)),
                               trace=_trace)
    _CACHE["last_result"] = res
    Yp = np.concatenate([r["YPOUT"] for r in res.results], axis=0)
    X = np.concatenate([r["XOUT"] for r in res.results], axis=0)
    return (Yp, X)


# revision 17
# speedup vs baseline: 1.0030x; 1.0030x over previous
"""Batch-OMP DictionaryLearning kernel for 8x Trainium2 NeuronCores.

Per core (data-parallel over batch, BLOC=2048 items = 16 tiles of 128):
  G = D @ D.T + eps*I, augmented with its diagonal as a 513th column and
  stored row-major in DRAM for per-item row gathers.
  hbar = Y @ D.T via PE (fp32).
  Incremental Cholesky-OMP (c-form), K=8 iterations:
    argmax |h| via h^2 + InstMax/InstMaxIndex (first-match tie semantics,
    matching jnp.argmax), indirect-DMA gather of the selected G_aug row,
    w_j = c_j[idx] via onehot multiply-reduce, unnormalized
    c'_k = GD - sum_j (w_j*rc_j) c'_j, rank-1 update h -= (yk_k*rc_k) c'_k.
  Batched back-substitution solves L^T x = yk; X assembled on PE via
  diag(x_k) @ onehot_k matmuls; Y_pred = X @ D via PE transposes.

Tiles processed in 3 passes of [6,6,4] (SBUF capacity for c' planes).
"""
import numpy as np

B, M, N, K = 16384, 256, 512, 8
NCORES = 8
BLOC = B // NCORES           # 2048
NT = BLOC // 128             # 16
PASS_T = [6, 6, 4]
TMAX = 6
DIAG_EPS = 1e-5
ROWW = 516                   # padded G_aug row width (513 used)
P = 128
import os as _os
PE_K_MIN = int(_os.environ.get("PE_K_MIN", "3"))
POOL_TILES = int(_os.environ.get("POOL_TILES", "0"))
TINY_GROUPS = int(_os.environ.get("TINY_GROUPS", "6"))
HSQ_DVE = int(_os.environ.get("HSQ_DVE", "0"))
PHG_POOL = int(_os.environ.get("PHG_POOL", "0"))
POOL_JMOD = int(_os.environ.get("POOL_JMOD", "0"))

_CACHE = {}


def _build_nc():
    import concourse.bacc as bacc
    import concourse.mybir as mybir
    from concourse.alu_op_type import AluOpType as aop
    from concourse.bass import IndirectOffsetOnAxis
    from concourse.tile import TileContext

    F32 = mybir.dt.float32
    U32 = mybir.dt.uint32
    AF = mybir.ActivationFunctionType
    AX = mybir.AxisListType.X

    nc = bacc.Bacc(None, target_bir_lowering=False)

    Yd = nc.dram_tensor("Y", [BLOC, M], F32, kind="ExternalInput")
    Dd = nc.dram_tensor("D", [N, M], F32, kind="ExternalInput")
    IOTAd = nc.dram_tensor("IOTA", [P, N], F32, kind="ExternalInput")
    M1d = nc.dram_tensor("M1", [P, 896], F32, kind="ExternalInput")
    XOUT = nc.dram_tensor("XOUT", [BLOC, N], F32, kind="ExternalOutput")
    YPOUT = nc.dram_tensor("YPOUT", [BLOC, M], F32, kind="ExternalOutput")

    with TileContext(nc) as tc:
        with (
            tc.tile_pool(name="dram", bufs=1, space="DRAM") as dpool,
            tc.tile_pool(name="const", bufs=1) as cpool,
            tc.tile_pool(name="mats", bufs=1) as mpool,
            tc.tile_pool(name="state", bufs=1) as spool,
            tc.tile_pool(name="work", bufs=2) as wpool,
            tc.tile_pool(name="ps_t", bufs=2, space="PSUM") as pst,
            tc.tile_pool(name="ps_mm", bufs=2, space="PSUM") as psm,
        ):
            GAUG = dpool.tile([N, ROWW], F32)
            HBARD = dpool.tile([NT, P, N], F32)

            iota = cpool.tile([P, N], F32)
            m1 = cpool.tile([P, 896], F32)
            nc.sync.dma_start(iota[:], IOTAd[:])
            nc.sync.dma_start(m1[:], M1d[:])
            IDM = m1[:, 384:512]                     # [128,128] identity
            IDM_full = cpool.tile([P, P], F32)
            nc.vector.tensor_copy(IDM_full[:], IDM)

            # ---------------- phase A: G_aug + hbar ----------------
            D_sb = mpool.tile([P, 4, M], F32)        # D rows chunked
            nc.sync.dma_start(D_sb[:], Dd.rearrange("(c p) m -> p c m", p=P))
            Dt = mpool.tile([P, 2, N], F32)          # D^T chunked over m
            for nch in range(4):
                for mc in range(2):
                    tp = pst.tile([P, P], F32, tag="tp")
                    nc.tensor.transpose(
                        tp[:], D_sb[:, nch, mc * P:(mc + 1) * P], IDM)
                    nc.scalar.copy(Dt[:, mc, nch * P:(nch + 1) * P], tp[:])

            G_sb = mpool.tile([P, 4, N], F32)
            diagS = mpool.tile([P, 4], F32)
            junkG = wpool.tile([P, N], F32, tag="junkG")
            for c in range(4):
                gp = psm.tile([P, N], F32, tag="mm")
                for mc in range(2):
                    nc.tensor.matmul(
                        gp[:], Dt[:, mc, c * P:(c + 1) * P], Dt[:, mc, :],
                        start=(mc == 0), stop=(mc == 1))
                m1v = m1[:, 384 - c * P: 896 - c * P]
                nc.vector.scalar_tensor_tensor(
                    G_sb[:, c, :], m1v, DIAG_EPS, gp[:], aop.mult, aop.add)
                nc.vector.scalar_tensor_tensor(
                    junkG[:], G_sb[:, c, :], 1.0, m1v, aop.bypass, aop.mult,
                    accum_out=diagS[:, c:c + 1])
            gaug_v = GAUG.rearrange("(c p) w -> p c w", p=P)
            for c in range(4):
                nc.sync.dma_start(gaug_v[:, c, 0:N], G_sb[:, c, :])
                nc.sync.dma_start(gaug_v[:, c, N:N + 1], diagS[:, c:c + 1])

            Yv = Yd.rearrange("(g p) m -> g p m", p=P)
            for g in range(NT):
                yt = wpool.tile([P, M], F32, tag="yt")
                nc.sync.dma_start(yt[:], Yv[g])
                yT = wpool.tile([P, 2, P], F32, tag="yT")
                for mc in range(2):
                    tp = pst.tile([P, P], F32, tag="tp")
                    nc.tensor.transpose(tp[:], yt[:, mc * P:(mc + 1) * P], IDM)
                    nc.scalar.copy(yT[:, mc, :], tp[:])
                hp = psm.tile([P, N], F32, tag="mm")
                for mc in range(2):
                    nc.tensor.matmul(hp[:], yT[:, mc, :], Dt[:, mc, :],
                                     start=(mc == 0), stop=(mc == 1))
                hb = wpool.tile([P, N], F32, tag="hb")
                nc.scalar.copy(hb[:], hp[:])
                nc.sync.dma_start(HBARD[g], hb[:])

            # ---------------- OMP state ----------------
            h_sb = spool.tile([P, TMAX, N], F32)
            cp_sb = spool.tile([P, TMAX, K, N], F32)
            GD = spool.tile([P, TMAX, ROWW], F32)
            hsq = spool.tile([P, 2, N], F32)
            onehot = spool.tile([P, 2, N], F32)
            mx8 = spool.tile([P, TMAX, 8], F32)
            ixu8 = spool.tile([P, NT, K, 8], U32)
            xsave = spool.tile([P, NT, K], F32)
            hsel = spool.tile([P, TMAX], F32)
            idx1f = spool.tile([P, TMAX], F32)
            wraw = spool.tile([P, TMAX, K], F32)
            wnrm = spool.tile([P, TMAX, K], F32)
            wsq3 = spool.tile([P, TMAX, K], F32)
            wsq = spool.tile([P, TMAX], F32)
            wc2 = spool.tile([P, TMAX], F32)
            wc = spool.tile([P, TMAX], F32)
            rcS = spool.tile([P, TMAX, K], F32)
            rcN = spool.tile([P, TMAX, K], F32)
            ykS = spool.tile([P, TMAX, K], F32)
            gS = spool.tile([P, TMAX, K], F32)
            hscal = spool.tile([P, TMAX], F32)
            LS = spool.tile([P, TMAX, K, K], F32)
            junk = spool.tile([P, N], F32)
            junk2 = spool.tile([P, N], F32)
            junk3 = spool.tile([P, N], F32)
            xacc = spool.tile([P, TMAX], F32)
            tmpT = spool.tile([P, TMAX], F32)

            XOUTv = XOUT.rearrange("(g p) n -> g p n", p=P)
            YPOUTv = YPOUT.rearrange("(g p) m -> g p m", p=P)

            g0 = 0
            for pi, T in enumerate(PASS_T):
                for t in range(T):
                    nc.sync.dma_start(h_sb[:, t, :], HBARD[g0 + t])
                for k in range(K):
                    ngr = max(1, min(TINY_GROUPS, T))
                    szs = [T // ngr + (1 if i < T % ngr else 0)
                           for i in range(ngr)]
                    gb0 = 0
                    for sz in szs:
                        ga, gb = gb0, gb0 + sz
                        gb0 = gb
                        # ---- stage A ----
                        for t in range(ga, gb):
                            g = g0 + t
                            hq = hsq[:, t % 2, :]
                            oh = onehot[:, t % 2, :]
                            if HSQ_DVE:
                                nc.vector.tensor_mul(hq, h_sb[:, t, :],
                                                     h_sb[:, t, :])
                            else:
                                nc.scalar.activation(hq, h_sb[:, t, :],
                                                     AF.Square)
                            nc.vector.max(mx8[:, t, :], hq)
                            nc.vector.max_index(ixu8[:, g, k, :],
                                                mx8[:, t, :], hq)
                            nc.gpsimd.indirect_dma_start(
                                out=GD[:, t, :], out_offset=None,
                                in_=GAUG[:, :],
                                in_offset=IndirectOffsetOnAxis(
                                    ap=ixu8[:, g, k, 0:1], axis=0))
                            nc.vector.tensor_copy(idx1f[:, t:t + 1],
                                                  ixu8[:, g, k, 0:1])
                            nc.vector.tensor_scalar(
                                oh, iota[:], idx1f[:, t:t + 1], None,
                                aop.is_equal)
                            seng = nc.gpsimd if t < POOL_TILES else nc.vector
                            seng.scalar_tensor_tensor(
                                junk[:], oh, 1.0, h_sb[:, t, :],
                                aop.bypass, aop.mult,
                                accum_out=hsel[:, t:t + 1])
                            for j in range(k):
                                jeng = (nc.gpsimd if (POOL_JMOD and
                                        j % POOL_JMOD == POOL_JMOD - 1)
                                        else seng)
                                jeng.scalar_tensor_tensor(
                                    junk2[:] if jeng is nc.vector else junk3[:],
                                    cp_sb[:, t, j, :], 1.0, oh,
                                    aop.bypass, aop.mult,
                                    accum_out=wraw[:, t, j:j + 1])
                        # ---- batched tiny algebra (group slice) ----
                        if k > 0:
                            nc.vector.tensor_mul(
                                wnrm[:, ga:gb, 0:k], wraw[:, ga:gb, 0:k],
                                rcS[:, ga:gb, 0:k])
                            nc.vector.tensor_mul(
                                wsq3[:, ga:gb, 0:k], wnrm[:, ga:gb, 0:k],
                                wnrm[:, ga:gb, 0:k])
                            nc.vector.tensor_reduce(
                                wsq[:, ga:gb], wsq3[:, ga:gb, 0:k], AX,
                                aop.add)
                            nc.vector.tensor_sub(
                                wc2[:, ga:gb], GD[:, ga:gb, N], wsq[:, ga:gb])
                        else:
                            nc.vector.tensor_copy(wc2[:, ga:gb],
                                                  GD[:, ga:gb, N])
                        nc.vector.tensor_scalar_max(
                            wc2[:, ga:gb], wc2[:, ga:gb], DIAG_EPS)
                        nc.scalar.activation(wc[:, ga:gb], wc2[:, ga:gb],
                                             AF.Sqrt)
                        nc.vector.reciprocal(rcS[:, ga:gb, k], wc[:, ga:gb])
                        nc.vector.tensor_scalar_mul(
                            rcN[:, ga:gb, k], rcS[:, ga:gb, k], -1.0)
                        nc.vector.tensor_mul(
                            ykS[:, ga:gb, k], hsel[:, ga:gb],
                            rcS[:, ga:gb, k])
                        nc.vector.tensor_mul(
                            hscal[:, ga:gb], ykS[:, ga:gb, k],
                            rcN[:, ga:gb, k])
                        if k > 0:
                            nc.vector.tensor_mul(
                                gS[:, ga:gb, 0:k], wnrm[:, ga:gb, 0:k],
                                rcN[:, ga:gb, 0:k])
                            nc.vector.tensor_copy(
                                LS[:, ga:gb, k, 0:k], wnrm[:, ga:gb, 0:k])
                        # ---- stage C ----
                        for t in range(ga, gb):
                            if k == 0:
                                nc.scalar.copy(cp_sb[:, t, 0, :],
                                               GD[:, t, 0:N])
                            elif k >= PE_K_MIN:
                                cps = psm.tile([P, N], F32, tag="mm")
                                nc.tensor.matmul(cps[:], IDM_full[:],
                                                 GD[:, t, 0:N],
                                                 start=True, stop=False)
                                for j in range(k):
                                    dgj = wpool.tile([P, P], F32, tag="dgj")
                                    nc.scalar.activation(
                                        dgj[:], IDM, AF.Copy,
                                        scale=gS[:, t, j:j + 1])
                                    nc.tensor.matmul(cps[:], dgj[:],
                                                     cp_sb[:, t, j, :],
                                                     start=False,
                                                     stop=(j == k - 1))
                                nc.scalar.copy(cp_sb[:, t, k, :], cps[:])
                            else:
                                nc.vector.scalar_tensor_tensor(
                                    cp_sb[:, t, k, :], cp_sb[:, t, 0, :],
                                    gS[:, t, 0:1], GD[:, t, 0:N],
                                    aop.mult, aop.add)
                                for j in range(1, k):
                                    nc.vector.scalar_tensor_tensor(
                                        cp_sb[:, t, k, :], cp_sb[:, t, j, :],
                                        gS[:, t, j:j + 1], cp_sb[:, t, k, :],
                                        aop.mult, aop.add)
                            nc.vector.scalar_tensor_tensor(
                                h_sb[:, t, :], cp_sb[:, t, k, :],
                                hscal[:, t:t + 1], h_sb[:, t, :],
                                aop.mult, aop.add)
                # ---- back-substitution: L^T x = yk ----
                for i in range(K - 1, -1, -1):
                    nc.vector.tensor_copy(xacc[:, 0:T], ykS[:, 0:T, i])
                    for j in range(i + 1, K):
                        nc.vector.tensor_mul(
                            tmpT[:, 0:T], LS[:, 0:T, j, i],
                            xsave[:, g0:g0 + T, j])
                        nc.vector.tensor_sub(
                            xacc[:, 0:T], xacc[:, 0:T], tmpT[:, 0:T])
                    nc.vector.tensor_mul(
                        xsave[:, g0:g0 + T, i], xacc[:, 0:T], rcS[:, 0:T, i])
                # ---- phase G for this pass: X + Y_pred ----
                for t in range(T):
                    g = g0 + t
                    idxf = wpool.tile([P, K], F32, tag="idxf")
                    nc.vector.tensor_copy(idxf[:], ixu8[:, g, :, 0])
                    Xps = psm.tile([P, N], F32, tag="mm")
                    for k in range(K):
                        ohg = wpool.tile([P, N], F32, tag="ohg")
                        oeng = nc.gpsimd if PHG_POOL else nc.vector
                        oeng.tensor_scalar(
                            ohg[:], iota[:], idxf[:, k:k + 1], None,
                            aop.is_equal)
                        dgx = wpool.tile([P, P], F32, tag="dgx")
                        nc.scalar.activation(
                            dgx[:], IDM, AF.Copy, scale=xsave[:, g, k:k + 1])
                        nc.tensor.matmul(Xps[:], dgx[:], ohg[:],
                                         start=(k == 0), stop=(k == K - 1))
                    Xsb = wpool.tile([P, N], F32, tag="Xsb")
                    nc.scalar.copy(Xsb[:], Xps[:])
                    nc.sync.dma_start(XOUTv[g], Xsb[:])
                    Yps = psm.tile([P, M], F32, tag="mm")
                    for c in range(4):
                        tp = pst.tile([P, P], F32, tag="tp")
                        nc.tensor.transpose(
                            tp[:], Xsb[:, c * P:(c + 1) * P], IDM)
                        xT = wpool.tile([P, P], F32, tag="xT")
                        nc.scalar.copy(xT[:], tp[:])
                        nc.tensor.matmul(Yps[:], xT[:], D_sb[:, c, :],
                                         start=(c == 0), stop=(c == 3))
                    Ysb = wpool.tile([P, M], F32, tag="Ysb")
                    nc.scalar.copy(Ysb[:], Yps[:])
                    nc.sync.dma_start(YPOUTv[g], Ysb[:])
                g0 += T
    nc.compile()
    return nc


def _get_nc():
    if "nc" not in _CACHE:
        _CACHE["nc"] = _build_nc()
    return _CACHE["nc"]


def _host_consts():
    iota = np.tile(np.arange(N, dtype=np.float32), (P, 1))
    m1 = np.zeros((P, 896), np.float32)
    m1[np.arange(P), 384 + np.arange(P)] = 1.0
    return iota, m1


def kernel(Y, D, _trace=False):
    from concourse.bass_utils import run_bass_kernel_spmd

    Y = np.ascontiguousarray(Y, dtype=np.float32)
    D = np.ascontiguousarray(D, dtype=np.float32)
    nc = _get_nc()
    iota, m1 = _host_consts()
    in_maps = []
    for c in range(NCORES):
        in_maps.append({
            "Y": Y[c * BLOC:(c + 1) * BLOC],
            "D": D,
            "IOTA": iota,
            "M1": m1,
        })
    res = run_bass_kernel_spmd(nc, in_maps, list(range(NCORES)),
                               trace=_trace)
    _CACHE["last_result"] = res
    Yp = np.concatenate([r["YPOUT"] for r in res.results], axis=0)
    X = np.concatenate([r["XOUT"] for r in res.results], axis=0)
    return (Yp, X)


# revision 19
# speedup vs baseline: 1.0199x; 1.0169x over previous
"""Batch-OMP DictionaryLearning kernel for 8x Trainium2 NeuronCores.

Per core (data-parallel over batch, BLOC=2048 items = 16 tiles of 128):
  G = D @ D.T + eps*I, augmented with its diagonal as a 513th column and
  stored row-major in DRAM for per-item row gathers.
  hbar = Y @ D.T via PE (fp32).
  Incremental Cholesky-OMP (c-form), K=8 iterations:
    argmax |h| via h^2 + InstMax/InstMaxIndex (first-match tie semantics,
    matching jnp.argmax), indirect-DMA gather of the selected G_aug row,
    w_j = c_j[idx] via onehot multiply-reduce, unnormalized
    c'_k = GD - sum_j (w_j*rc_j) c'_j, rank-1 update h -= (yk_k*rc_k) c'_k.
  Batched back-substitution solves L^T x = yk; X assembled on PE via
  diag(x_k) @ onehot_k matmuls; Y_pred = X @ D via PE transposes.

Tiles processed in 3 passes of [6,6,4] (SBUF capacity for c' planes).
"""
import numpy as np

B, M, N, K = 16384, 256, 512, 8
NCORES = 8
BLOC = B // NCORES           # 2048
NT = BLOC // 128             # 16
PASS_T = [6, 6, 4]
TMAX = 6
DIAG_EPS = 1e-5
ROWW = 516                   # padded G_aug row width (513 used)
P = 128
import os as _os
PE_K_MIN = int(_os.environ.get("PE_K_MIN", "3"))
POOL_TILES = int(_os.environ.get("POOL_TILES", "0"))
TINY_GROUPS = int(_os.environ.get("TINY_GROUPS", "6"))
HSQ_DVE = int(_os.environ.get("HSQ_DVE", "0"))
PHG_POOL = int(_os.environ.get("PHG_POOL", "0"))
POOL_JMOD = int(_os.environ.get("POOL_JMOD", "0"))

_CACHE = {}


def _build_nc():
    import concourse.bacc as bacc
    import concourse.mybir as mybir
    from concourse.alu_op_type import AluOpType as aop
    from concourse.bass import IndirectOffsetOnAxis
    from concourse.tile import TileContext

    F32 = mybir.dt.float32
    U32 = mybir.dt.uint32
    AF = mybir.ActivationFunctionType
    AX = mybir.AxisListType.X

    nc = bacc.Bacc(None, target_bir_lowering=False)

    Yd = nc.dram_tensor("Y", [BLOC, M], F32, kind="ExternalInput")
    Dd = nc.dram_tensor("D", [N, M], F32, kind="ExternalInput")
    IOTAd = nc.dram_tensor("IOTA", [P, N], F32, kind="ExternalInput")
    M1d = nc.dram_tensor("M1", [P, 896], F32, kind="ExternalInput")
    XOUT = nc.dram_tensor("XOUT", [BLOC, N], F32, kind="ExternalOutput")
    YPOUT = nc.dram_tensor("YPOUT", [BLOC, M], F32, kind="ExternalOutput")

    with TileContext(nc) as tc:
        with (
            tc.tile_pool(name="dram", bufs=1, space="DRAM") as dpool,
            tc.tile_pool(name="const", bufs=1) as cpool,
            tc.tile_pool(name="mats", bufs=1) as mpool,
            tc.tile_pool(name="state", bufs=1) as spool,
            tc.tile_pool(name="work", bufs=2) as wpool,
            tc.tile_pool(name="ps_t", bufs=2, space="PSUM") as pst,
            tc.tile_pool(name="ps_mm", bufs=2, space="PSUM") as psm,
        ):
            GAUG = dpool.tile([N, ROWW], F32)
            HBARD = dpool.tile([NT, P, N], F32)

            iota = cpool.tile([P, N], F32)
            m1 = cpool.tile([P, 896], F32)
            nc.sync.dma_start(iota[:], IOTAd[:])
            nc.sync.dma_start(m1[:], M1d[:])
            IDM = m1[:, 384:512]                     # [128,128] identity
            IDM_full = cpool.tile([P, P], F32)
            nc.vector.tensor_copy(IDM_full[:], IDM)

            # ---------------- phase A: G_aug + hbar ----------------
            D_sb = mpool.tile([P, 4, M], F32)        # D rows chunked
            nc.sync.dma_start(D_sb[:], Dd.rearrange("(c p) m -> p c m", p=P))
            Dt = mpool.tile([P, 2, N], F32)          # D^T chunked over m
            for nch in range(4):
                for mc in range(2):
                    tp = pst.tile([P, P], F32, tag="tp")
                    nc.tensor.transpose(
                        tp[:], D_sb[:, nch, mc * P:(mc + 1) * P], IDM)
                    nc.scalar.copy(Dt[:, mc, nch * P:(nch + 1) * P], tp[:])

            G_sb = mpool.tile([P, 4, N], F32)
            diagS = mpool.tile([P, 4], F32)
            junkG = wpool.tile([P, N], F32, tag="junkG")
            for c in range(4):
                gp = psm.tile([P, N], F32, tag="mm")
                for mc in range(2):
                    nc.tensor.matmul(
                        gp[:], Dt[:, mc, c * P:(c + 1) * P], Dt[:, mc, :],
                        start=(mc == 0), stop=(mc == 1))
                m1v = m1[:, 384 - c * P: 896 - c * P]
                nc.vector.scalar_tensor_tensor(
                    G_sb[:, c, :], m1v, DIAG_EPS, gp[:], aop.mult, aop.add)
                nc.vector.scalar_tensor_tensor(
                    junkG[:], G_sb[:, c, :], 1.0, m1v, aop.bypass, aop.mult,
                    accum_out=diagS[:, c:c + 1])
            gaug_v = GAUG.rearrange("(c p) w -> p c w", p=P)
            for c in range(4):
                nc.sync.dma_start(gaug_v[:, c, 0:N], G_sb[:, c, :])
                nc.sync.dma_start(gaug_v[:, c, N:N + 1], diagS[:, c:c + 1])

            h_sb = spool.tile([P, TMAX, N], F32)
            Yv = Yd.rearrange("(g p) m -> g p m", p=P)
            for g in range(NT):
                yt = wpool.tile([P, M], F32, tag="yt")
                nc.sync.dma_start(yt[:], Yv[g])
                yT = wpool.tile([P, 2, P], F32, tag="yT")
                for mc in range(2):
                    tp = pst.tile([P, P], F32, tag="tp")
                    nc.tensor.transpose(tp[:], yt[:, mc * P:(mc + 1) * P], IDM)
                    nc.scalar.copy(yT[:, mc, :], tp[:])
                hp = psm.tile([P, N], F32, tag="mm")
                for mc in range(2):
                    nc.tensor.matmul(hp[:], yT[:, mc, :], Dt[:, mc, :],
                                     start=(mc == 0), stop=(mc == 1))
                hb = wpool.tile([P, N], F32, tag="hb")
                nc.scalar.copy(hb[:], hp[:])
                if g < PASS_T[0]:
                    nc.scalar.copy(h_sb[:, g, :], hp[:])
                else:
                    nc.sync.dma_start(HBARD[g], hb[:])

            # ---------------- OMP state ----------------
            cp_sb = spool.tile([P, TMAX, K, N], F32)
            GD = spool.tile([P, TMAX, ROWW], F32)
            hsq = spool.tile([P, 2, N], F32)
            onehot = spool.tile([P, 2, N], F32)
            mx8 = spool.tile([P, TMAX, 8], F32)
            ixu8 = spool.tile([P, NT, K, 8], U32)
            xsave = spool.tile([P, NT, K], F32)
            hsel = spool.tile([P, TMAX], F32)
            idx1f = spool.tile([P, TMAX], F32)
            wraw = spool.tile([P, TMAX, K], F32)
            wnrm = spool.tile([P, TMAX, K], F32)
            wsq3 = spool.tile([P, TMAX, K], F32)
            wsq = spool.tile([P, TMAX], F32)
            wc2 = spool.tile([P, TMAX], F32)
            wc = spool.tile([P, TMAX], F32)
            rcS = spool.tile([P, TMAX, K], F32)
            rcN = spool.tile([P, TMAX, K], F32)
            ykS = spool.tile([P, TMAX, K], F32)
            gS = spool.tile([P, TMAX, K], F32)
            hscal = spool.tile([P, TMAX], F32)
            LS = spool.tile([P, TMAX, K, K], F32)
            junk = spool.tile([P, N], F32)
            junk2 = spool.tile([P, N], F32)
            junk3 = spool.tile([P, N], F32)
            xacc = spool.tile([P, TMAX], F32)
            tmpT = spool.tile([P, TMAX], F32)

            XOUTv = XOUT.rearrange("(g p) n -> g p n", p=P)
            YPOUTv = YPOUT.rearrange("(g p) m -> g p m", p=P)

            g0 = 0
            for pi, T in enumerate(PASS_T):
                for t in range(T):
                    if pi > 0:
                        nc.sync.dma_start(h_sb[:, t, :], HBARD[g0 + t])
                for k in range(K):
                    ngr = max(1, min(TINY_GROUPS, T))
                    szs = [T // ngr + (1 if i < T % ngr else 0)
                           for i in range(ngr)]
                    gb0 = 0
                    for sz in szs:
                        ga, gb = gb0, gb0 + sz
                        gb0 = gb
                        # ---- stage A ----
                        for t in range(ga, gb):
                            g = g0 + t
                            hq = hsq[:, t % 2, :]
                            oh = onehot[:, t % 2, :]
                            if HSQ_DVE:
                                nc.vector.tensor_mul(hq, h_sb[:, t, :],
                                                     h_sb[:, t, :])
                            else:
                                nc.scalar.activation(hq, h_sb[:, t, :],
                                                     AF.Square)
                            nc.vector.max(mx8[:, t, :], hq)
                            nc.vector.max_index(ixu8[:, g, k, :],
                                                mx8[:, t, :], hq)
                            nc.gpsimd.indirect_dma_start(
                                out=GD[:, t, :], out_offset=None,
                                in_=GAUG[:, :],
                                in_offset=IndirectOffsetOnAxis(
                                    ap=ixu8[:, g, k, 0:1], axis=0))
                            nc.vector.tensor_copy(idx1f[:, t:t + 1],
                                                  ixu8[:, g, k, 0:1])
                            nc.vector.tensor_scalar(
                                oh, iota[:], idx1f[:, t:t + 1], None,
                                aop.is_equal)
                            seng = nc.gpsimd if t < POOL_TILES else nc.vector
                            seng.scalar_tensor_tensor(
                                junk[:], oh, 1.0, h_sb[:, t, :],
                                aop.bypass, aop.mult,
                                accum_out=hsel[:, t:t + 1])
                            for j in range(k):
                                jeng = (nc.gpsimd if (POOL_JMOD and
                                        j % POOL_JMOD == POOL_JMOD - 1)
                                        else seng)
                                jeng.scalar_tensor_tensor(
                                    junk2[:] if jeng is nc.vector else junk3[:],
                                    cp_sb[:, t, j, :], 1.0, oh,
                                    aop.bypass, aop.mult,
                                    accum_out=wraw[:, t, j:j + 1])
                        # ---- batched tiny algebra (group slice) ----
                        if k > 0:
                            nc.vector.tensor_mul(
                                wnrm[:, ga:gb, 0:k], wraw[:, ga:gb, 0:k],
                                rcS[:, ga:gb, 0:k])
                            nc.vector.tensor_mul(
                                wsq3[:, ga:gb, 0:k], wnrm[:, ga:gb, 0:k],
                                wnrm[:, ga:gb, 0:k])
                            nc.vector.tensor_reduce(
                                wsq[:, ga:gb], wsq3[:, ga:gb, 0:k], AX,
                                aop.add)
                            nc.vector.tensor_sub(
                                wc2[:, ga:gb], GD[:, ga:gb, N], wsq[:, ga:gb])
                        else:
                            nc.vector.tensor_copy(wc2[:, ga:gb],
                                                  GD[:, ga:gb, N])
                        nc.vector.tensor_scalar_max(
                            wc2[:, ga:gb], wc2[:, ga:gb], DIAG_EPS)
                        nc.scalar.activation(wc[:, ga:gb], wc2[:, ga:gb],
                                             AF.Sqrt)
                        nc.vector.reciprocal(rcS[:, ga:gb, k], wc[:, ga:gb])
                        nc.vector.tensor_scalar_mul(
                            rcN[:, ga:gb, k], rcS[:, ga:gb, k], -1.0)
                        nc.vector.tensor_mul(
                            ykS[:, ga:gb, k], hsel[:, ga:gb],
                            rcS[:, ga:gb, k])
                        if k < K - 1:
                            nc.vector.tensor_mul(
                                hscal[:, ga:gb], ykS[:, ga:gb, k],
                                rcN[:, ga:gb, k])
                        if k > 0:
                            nc.vector.tensor_mul(
                                gS[:, ga:gb, 0:k], wnrm[:, ga:gb, 0:k],
                                rcN[:, ga:gb, 0:k])
                            nc.vector.tensor_copy(
                                LS[:, ga:gb, k, 0:k], wnrm[:, ga:gb, 0:k])
                        # ---- stage C ----
                        for t in range(ga, gb):
                            if k == K - 1:
                                continue
                            if k == 0:
                                nc.scalar.copy(cp_sb[:, t, 0, :],
                                               GD[:, t, 0:N])
                            elif k >= PE_K_MIN:
                                cps = psm.tile([P, N], F32, tag="mm")
                                nc.tensor.matmul(cps[:], IDM_full[:],
                                                 GD[:, t, 0:N],
                                                 start=True, stop=False)
                                for j in range(k):
                                    dgj = wpool.tile([P, P], F32, tag="dgj")
                                    nc.scalar.activation(
                                        dgj[:], IDM, AF.Copy,
                                        scale=gS[:, t, j:j + 1])
                                    nc.tensor.matmul(cps[:], dgj[:],
                                                     cp_sb[:, t, j, :],
                                                     start=False,
                                                     stop=(j == k - 1))
                                nc.scalar.copy(cp_sb[:, t, k, :], cps[:])
                            else:
                                nc.vector.scalar_tensor_tensor(
                                    cp_sb[:, t, k, :], cp_sb[:, t, 0, :],
                                    gS[:, t, 0:1], GD[:, t, 0:N],
                                    aop.mult, aop.add)
                                for j in range(1, k):
                                    nc.vector.scalar_tensor_tensor(
                                        cp_sb[:, t, k, :], cp_sb[:, t, j, :],
                                        gS[:, t, j:j + 1], cp_sb[:, t, k, :],
                                        aop.mult, aop.add)
                            nc.vector.scalar_tensor_tensor(
                                h_sb[:, t, :], cp_sb[:, t, k, :],
                                hscal[:, t:t + 1], h_sb[:, t, :],
                                aop.mult, aop.add)
                # ---- back-substitution: L^T x = yk ----
                for i in range(K - 1, -1, -1):
                    nc.vector.tensor_copy(xacc[:, 0:T], ykS[:, 0:T, i])
                    for j in range(i + 1, K):
                        nc.vector.tensor_mul(
                            tmpT[:, 0:T], LS[:, 0:T, j, i],
                            xsave[:, g0:g0 + T, j])
                        nc.vector.tensor_sub(
                            xacc[:, 0:T], xacc[:, 0:T], tmpT[:, 0:T])
                    nc.vector.tensor_mul(
                        xsave[:, g0:g0 + T, i], xacc[:, 0:T], rcS[:, 0:T, i])
                # ---- phase G for this pass: X + Y_pred ----
                for t in range(T):
                    g = g0 + t
                    idxf = wpool.tile([P, K], F32, tag="idxf")
                    nc.vector.tensor_copy(idxf[:], ixu8[:, g, :, 0])
                    Xps = psm.tile([P, N], F32, tag="mm")
                    for k in range(K):
                        ohg = wpool.tile([P, N], F32, tag="ohg")
                        oeng = nc.gpsimd if PHG_POOL else nc.vector
                        oeng.tensor_scalar(
                            ohg[:], iota[:], idxf[:, k:k + 1], None,
                            aop.is_equal)
                        dgx = wpool.tile([P, P], F32, tag="dgx")
                        nc.scalar.activation(
                            dgx[:], IDM, AF.Copy, scale=xsave[:, g, k:k + 1])
                        nc.tensor.matmul(Xps[:], dgx[:], ohg[:],
                                         start=(k == 0), stop=(k == K - 1))
                    Xsb = wpool.tile([P, N], F32, tag="Xsb")
                    nc.scalar.copy(Xsb[:], Xps[:])
                    nc.sync.dma_start(XOUTv[g], Xsb[:])
                    Yps = psm.tile([P, M], F32, tag="mm")
                    for c in range(4):
                        tp = pst.tile([P, P], F32, tag="tp")
                        nc.tensor.transpose(
                            tp[:], Xsb[:, c * P:(c + 1) * P], IDM)
                        xT = wpool.tile([P, P], F32, tag="xT")
                        nc.scalar.copy(xT[:], tp[:])
                        nc.tensor.matmul(Yps[:], xT[:], D_sb[:, c, :],
                                         start=(c == 0), stop=(c == 3))
                    Ysb = wpool.tile([P, M], F32, tag="Ysb")
                    nc.scalar.copy(Ysb[:], Yps[:])
                    nc.sync.dma_start(YPOUTv[g], Ysb[:])
                g0 += T
    nc.compile()
    return nc


def _get_nc():
    if "nc" not in _CACHE:
        _CACHE["nc"] = _build_nc()
    return _CACHE["nc"]


def _host_consts():
    iota = np.tile(np.arange(N, dtype=np.float32), (P, 1))
    m1 = np.zeros((P, 896), np.float32)
    m1[np.arange(P), 384 + np.arange(P)] = 1.0
    return iota, m1


def kernel(Y, D, _trace=False):
    from concourse.bass_utils import run_bass_kernel_spmd

    Y = np.ascontiguousarray(Y, dtype=np.float32)
    D = np.ascontiguousarray(D, dtype=np.float32)
    nc = _get_nc()
    iota, m1 = _host_consts()
    in_maps = []
    for c in range(NCORES):
        in_maps.append({
            "Y": Y[c * BLOC:(c + 1) * BLOC],
            "D": D,
            "IOTA": iota,
            "M1": m1,
        })
    res = run_bass_kernel_spmd(nc, in_maps, list(range(NCORES)),
                               trace=_trace)
    _CACHE["last_result"] = res
    Yp = np.concatenate([r["YPOUT"] for r in res.results], axis=0)
    X = np.concatenate([r["XOUT"] for r in res.results], axis=0)
    return (Yp, X)


# revision 20
# speedup vs baseline: 1.0344x; 1.0142x over previous
"""Batch-OMP DictionaryLearning kernel for 8x Trainium2 NeuronCores.

Per core (data-parallel over batch, BLOC=2048 items = 16 tiles of 128):
  G = D @ D.T + eps*I, augmented with its diagonal as a 513th column and
  stored row-major in DRAM for per-item row gathers.
  hbar = Y @ D.T via PE (fp32).
  Incremental Cholesky-OMP (c-form), K=8 iterations:
    argmax |h| via h^2 + InstMax/InstMaxIndex (first-match tie semantics,
    matching jnp.argmax), indirect-DMA gather of the selected G_aug row,
    w_j = c_j[idx] via onehot multiply-reduce, unnormalized
    c'_k = GD - sum_j (w_j*rc_j) c'_j, rank-1 update h -= (yk_k*rc_k) c'_k.
  Batched back-substitution solves L^T x = yk; X assembled on PE via
  diag(x_k) @ onehot_k matmuls; Y_pred = X @ D via PE transposes.

Tiles processed in 3 passes of [6,6,4] (SBUF capacity for c' planes).
"""
import numpy as np

B, M, N, K = 16384, 256, 512, 8
NCORES = 8
BLOC = B // NCORES           # 2048
NT = BLOC // 128             # 16
PASS_T = [6, 6, 4]
TMAX = 6
DIAG_EPS = 1e-5
ROWW = 516                   # padded G_aug row width (513 used)
P = 128
import os as _os
PE_K_MIN = int(_os.environ.get("PE_K_MIN", "3"))
POOL_TILES = int(_os.environ.get("POOL_TILES", "0"))
TINY_GROUPS = int(_os.environ.get("TINY_GROUPS", "6"))
HSQ_DVE = int(_os.environ.get("HSQ_DVE", "0"))
PHG_POOL = int(_os.environ.get("PHG_POOL", "0"))
POOL_JMOD = int(_os.environ.get("POOL_JMOD", "0"))

_CACHE = {}


def _build_nc():
    import concourse.bacc as bacc
    import concourse.mybir as mybir
    from concourse.alu_op_type import AluOpType as aop
    from concourse.bass import IndirectOffsetOnAxis
    from concourse.tile import TileContext

    F32 = mybir.dt.float32
    U32 = mybir.dt.uint32
    AF = mybir.ActivationFunctionType
    AX = mybir.AxisListType.X

    nc = bacc.Bacc(None, target_bir_lowering=False)

    Yd = nc.dram_tensor("Y", [BLOC, M], F32, kind="ExternalInput")
    Dd = nc.dram_tensor("D", [N, M], F32, kind="ExternalInput")
    IOTAd = nc.dram_tensor("IOTA", [P, N], F32, kind="ExternalInput")
    M1d = nc.dram_tensor("M1", [P, 896], F32, kind="ExternalInput")
    XOUT = nc.dram_tensor("XOUT", [BLOC, N], F32, kind="ExternalOutput")
    YPOUT = nc.dram_tensor("YPOUT", [BLOC, M], F32, kind="ExternalOutput")

    with TileContext(nc) as tc:
        with (
            tc.tile_pool(name="dram", bufs=1, space="DRAM") as dpool,
            tc.tile_pool(name="const", bufs=1) as cpool,
            tc.tile_pool(name="mats", bufs=1) as mpool,
            tc.tile_pool(name="state", bufs=1) as spool,
            tc.tile_pool(name="work", bufs=2) as wpool,
            tc.tile_pool(name="ps_t", bufs=2, space="PSUM") as pst,
            tc.tile_pool(name="ps_mm", bufs=2, space="PSUM") as psm,
        ):
            GAUG = dpool.tile([N, ROWW], F32)
            HBARD = dpool.tile([NT, P, N], F32)

            iota = cpool.tile([P, N], F32)
            m1 = cpool.tile([P, 896], F32)
            nc.sync.dma_start(iota[:], IOTAd[:])
            nc.sync.dma_start(m1[:], M1d[:])
            IDM = m1[:, 384:512]                     # [128,128] identity
            IDM_full = cpool.tile([P, P], F32)
            nc.vector.tensor_copy(IDM_full[:], IDM)

            # ---------------- phase A: G_aug + hbar ----------------
            D_sb = mpool.tile([P, 4, M], F32)        # D rows chunked
            nc.sync.dma_start(D_sb[:], Dd.rearrange("(c p) m -> p c m", p=P))
            Dt = mpool.tile([P, 2, N], F32)          # D^T chunked over m
            for nch in range(4):
                for mc in range(2):
                    tp = pst.tile([P, P], F32, tag="tp")
                    nc.tensor.transpose(
                        tp[:], D_sb[:, nch, mc * P:(mc + 1) * P], IDM)
                    nc.scalar.copy(Dt[:, mc, nch * P:(nch + 1) * P], tp[:])

            G_sb = mpool.tile([P, 4, N], F32)
            diagS = mpool.tile([P, 4], F32)
            junkG = wpool.tile([P, N], F32, tag="junkG")
            for c in range(4):
                gp = psm.tile([P, N], F32, tag="mm")
                for mc in range(2):
                    nc.tensor.matmul(
                        gp[:], Dt[:, mc, c * P:(c + 1) * P], Dt[:, mc, :],
                        start=(mc == 0), stop=(mc == 1))
                m1v = m1[:, 384 - c * P: 896 - c * P]
                nc.vector.scalar_tensor_tensor(
                    G_sb[:, c, :], m1v, DIAG_EPS, gp[:], aop.mult, aop.add)
                nc.vector.scalar_tensor_tensor(
                    junkG[:], G_sb[:, c, :], 1.0, m1v, aop.bypass, aop.mult,
                    accum_out=diagS[:, c:c + 1])
            gaug_v = GAUG.rearrange("(c p) w -> p c w", p=P)
            for c in range(4):
                nc.sync.dma_start(gaug_v[:, c, 0:N], G_sb[:, c, :])
                nc.sync.dma_start(gaug_v[:, c, N:N + 1], diagS[:, c:c + 1])

            h_sb = spool.tile([P, TMAX, N], F32)
            Yv = Yd.rearrange("(g p) m -> g p m", p=P)
            for g in range(NT):
                yt = wpool.tile([P, M], F32, tag="yt")
                nc.sync.dma_start(yt[:], Yv[g])
                yT = wpool.tile([P, 2, P], F32, tag="yT")
                for mc in range(2):
                    tp = pst.tile([P, P], F32, tag="tp")
                    nc.tensor.transpose(tp[:], yt[:, mc * P:(mc + 1) * P], IDM)
                    nc.scalar.copy(yT[:, mc, :], tp[:])
                hp = psm.tile([P, N], F32, tag="mm")
                for mc in range(2):
                    nc.tensor.matmul(hp[:], yT[:, mc, :], Dt[:, mc, :],
                                     start=(mc == 0), stop=(mc == 1))
                hb = wpool.tile([P, N], F32, tag="hb")
                nc.scalar.copy(hb[:], hp[:])
                if g < PASS_T[0]:
                    nc.scalar.copy(h_sb[:, g, :], hp[:])
                else:
                    nc.sync.dma_start(HBARD[g], hb[:])

            # ---------------- OMP state ----------------
            cp_sb = spool.tile([P, TMAX, K, N], F32)
            GD = spool.tile([P, TMAX, ROWW], F32)
            hsq = spool.tile([P, 2, N], F32)
            onehot = spool.tile([P, 2, N], F32)
            mx8 = spool.tile([P, TMAX, 8], F32)
            ixu8 = spool.tile([P, NT, K, 8], U32)
            xsave = spool.tile([P, NT, K], F32)
            hsel = spool.tile([P, TMAX], F32)
            idx1f = spool.tile([P, TMAX], F32)
            wraw = spool.tile([P, TMAX, K], F32)
            wnrm = spool.tile([P, TMAX, K], F32)
            wsq3 = spool.tile([P, TMAX, K], F32)
            wsq = spool.tile([P, TMAX], F32)
            wc2 = spool.tile([P, TMAX], F32)
            wc = spool.tile([P, TMAX], F32)
            rcS = spool.tile([P, TMAX, K], F32)
            rcN = spool.tile([P, TMAX, K], F32)
            ykS = spool.tile([P, TMAX, K], F32)
            gS = spool.tile([P, TMAX, K], F32)
            hscal = spool.tile([P, TMAX], F32)
            LS = spool.tile([P, TMAX, K, K], F32)
            junk = spool.tile([P, N], F32)
            junk2 = spool.tile([P, N], F32)
            junk3 = spool.tile([P, N], F32)
            xacc = spool.tile([P, TMAX], F32)
            tmpT = spool.tile([P, TMAX], F32)

            XOUTv = XOUT.rearrange("(g p) n -> g p n", p=P)
            YPOUTv = YPOUT.rearrange("(g p) m -> g p m", p=P)

            g0 = 0
            for pi, T in enumerate(PASS_T):
                for t in range(T):
                    if pi > 0:
                        nc.sync.dma_start(h_sb[:, t, :], HBARD[g0 + t])
                for k in range(K):
                    ngr = max(1, min(TINY_GROUPS, T))
                    szs = [T // ngr + (1 if i < T % ngr else 0)
                           for i in range(ngr)]
                    gb0 = 0
                    for sz in szs:
                        ga, gb = gb0, gb0 + sz
                        gb0 = gb
                        # ---- stage A ----
                        for t in range(ga, gb):
                            g = g0 + t
                            hq = hsq[:, t % 2, :]
                            oh = onehot[:, t % 2, :]
                            if HSQ_DVE:
                                nc.vector.tensor_mul(hq, h_sb[:, t, :],
                                                     h_sb[:, t, :])
                            else:
                                nc.scalar.activation(hq, h_sb[:, t, :],
                                                     AF.Square)
                            nc.vector.max(mx8[:, t, :], hq)
                            nc.vector.max_index(ixu8[:, g, k, :],
                                                mx8[:, t, :], hq)
                            nc.gpsimd.indirect_dma_start(
                                out=GD[:, t, :], out_offset=None,
                                in_=GAUG[:, :],
                                in_offset=IndirectOffsetOnAxis(
                                    ap=ixu8[:, g, k, 0:1], axis=0))
                            nc.vector.tensor_copy(idx1f[:, t:t + 1],
                                                  ixu8[:, g, k, 0:1])
                            nc.vector.tensor_scalar(
                                oh, iota[:], idx1f[:, t:t + 1], None,
                                aop.is_equal)
                            seng = nc.gpsimd if t < POOL_TILES else nc.vector
                            seng.scalar_tensor_tensor(
                                junk[:], oh, 1.0, h_sb[:, t, :],
                                aop.bypass, aop.mult,
                                accum_out=hsel[:, t:t + 1])
                            for j in range(k):
                                jeng = (nc.gpsimd if (POOL_JMOD and
                                        j % POOL_JMOD == POOL_JMOD - 1)
                                        else seng)
                                jeng.scalar_tensor_tensor(
                                    junk2[:] if jeng is nc.vector else junk3[:],
                                    cp_sb[:, t, j, :], 1.0, oh,
                                    aop.bypass, aop.mult,
                                    accum_out=wraw[:, t, j:j + 1])
                        # ---- batched tiny algebra (group slice) ----
                        assert gb == ga + 1
                        t0 = ga
                        if k > 0:
                            # wnrm written straight into the L row
                            nc.vector.tensor_mul(
                                LS[:, t0, k, 0:k], wraw[:, ga:gb, 0:k],
                                rcS[:, ga:gb, 0:k])
                            # wsq = sum(wnrm^2) fused via STT self-mult accum
                            nc.vector.scalar_tensor_tensor(
                                wsq3[:, t0, 0:k], LS[:, t0, k, 0:k], 1.0,
                                LS[:, t0, k, 0:k], aop.bypass, aop.mult,
                                accum_out=wsq[:, ga:gb])
                            # wc2n = min(wsq - diag, -eps)  (= -max(diag-wsq, eps))
                            nc.vector.tensor_scalar(
                                wc2[:, ga:gb], wsq[:, ga:gb],
                                GD[:, t0, N:N + 1], -DIAG_EPS,
                                aop.subtract, aop.min)
                        else:
                            # wc2n = min(-diag, -eps)
                            nc.vector.tensor_scalar(
                                wc2[:, ga:gb], GD[:, t0, N:N + 1], -1.0,
                                -DIAG_EPS, aop.mult, aop.min)
                        nc.scalar.activation(wc[:, ga:gb], wc2[:, ga:gb],
                                             AF.Sqrt, scale=-1.0)
                        nc.vector.reciprocal(rcS[:, ga:gb, k], wc[:, ga:gb])
                        nc.vector.tensor_scalar_mul(
                            rcN[:, ga:gb, k], rcS[:, ga:gb, k], -1.0)
                        nc.vector.tensor_mul(
                            ykS[:, ga:gb, k], hsel[:, ga:gb],
                            rcS[:, ga:gb, k])
                        if k < K - 1:
                            nc.vector.tensor_mul(
                                hscal[:, ga:gb], ykS[:, ga:gb, k],
                                rcN[:, ga:gb, k])
                        if k > 0:
                            nc.vector.tensor_mul(
                                gS[:, ga:gb, 0:k], LS[:, t0, k, 0:k],
                                rcN[:, ga:gb, 0:k])
                        # ---- stage C ----
                        for t in range(ga, gb):
                            if k == K - 1:
                                continue
                            if k == 0:
                                nc.scalar.copy(cp_sb[:, t, 0, :],
                                               GD[:, t, 0:N])
                            elif k >= PE_K_MIN:
                                cps = psm.tile([P, N], F32, tag="mm")
                                nc.tensor.matmul(cps[:], IDM_full[:],
                                                 GD[:, t, 0:N],
                                                 start=True, stop=False)
                                for j in range(k):
                                    dgj = wpool.tile([P, P], F32, tag="dgj")
                                    nc.scalar.activation(
                                        dgj[:], IDM, AF.Copy,
                                        scale=gS[:, t, j:j + 1])
                                    nc.tensor.matmul(cps[:], dgj[:],
                                                     cp_sb[:, t, j, :],
                                                     start=False,
                                                     stop=(j == k - 1))
                                nc.scalar.copy(cp_sb[:, t, k, :], cps[:])
                            else:
                                nc.vector.scalar_tensor_tensor(
                                    cp_sb[:, t, k, :], cp_sb[:, t, 0, :],
                                    gS[:, t, 0:1], GD[:, t, 0:N],
                                    aop.mult, aop.add)
                                for j in range(1, k):
                                    nc.vector.scalar_tensor_tensor(
                                        cp_sb[:, t, k, :], cp_sb[:, t, j, :],
                                        gS[:, t, j:j + 1], cp_sb[:, t, k, :],
                                        aop.mult, aop.add)
                            nc.vector.scalar_tensor_tensor(
                                h_sb[:, t, :], cp_sb[:, t, k, :],
                                hscal[:, t:t + 1], h_sb[:, t, :],
                                aop.mult, aop.add)
                # ---- back-substitution: L^T x = yk ----
                for i in range(K - 1, -1, -1):
                    nc.vector.tensor_copy(xacc[:, 0:T], ykS[:, 0:T, i])
                    for j in range(i + 1, K):
                        nc.vector.tensor_mul(
                            tmpT[:, 0:T], LS[:, 0:T, j, i],
                            xsave[:, g0:g0 + T, j])
                        nc.vector.tensor_sub(
                            xacc[:, 0:T], xacc[:, 0:T], tmpT[:, 0:T])
                    nc.vector.tensor_mul(
                        xsave[:, g0:g0 + T, i], xacc[:, 0:T], rcS[:, 0:T, i])
                # ---- phase G for this pass: X + Y_pred ----
                for t in range(T):
                    g = g0 + t
                    idxf = wpool.tile([P, K], F32, tag="idxf")
                    nc.vector.tensor_copy(idxf[:], ixu8[:, g, :, 0])
                    Xps = psm.tile([P, N], F32, tag="mm")
                    for k in range(K):
                        ohg = wpool.tile([P, N], F32, tag="ohg")
                        oeng = nc.gpsimd if PHG_POOL else nc.vector
                        oeng.tensor_scalar(
                            ohg[:], iota[:], idxf[:, k:k + 1], None,
                            aop.is_equal)
                        dgx = wpool.tile([P, P], F32, tag="dgx")
                        nc.scalar.activation(
                            dgx[:], IDM, AF.Copy, scale=xsave[:, g, k:k + 1])
                        nc.tensor.matmul(Xps[:], dgx[:], ohg[:],
                                         start=(k == 0), stop=(k == K - 1))
                    Xsb = wpool.tile([P, N], F32, tag="Xsb")
                    nc.scalar.copy(Xsb[:], Xps[:])
                    nc.sync.dma_start(XOUTv[g], Xsb[:])
                    Yps = psm.tile([P, M], F32, tag="mm")
                    for c in range(4):
                        tp = pst.tile([P, P], F32, tag="tp")
                        nc.tensor.transpose(
                            tp[:], Xsb[:, c * P:(c + 1) * P], IDM)
                        xT = wpool.tile([P, P], F32, tag="xT")
                        nc.scalar.copy(xT[:], tp[:])
                        nc.tensor.matmul(Yps[:], xT[:], D_sb[:, c, :],
                                         start=(c == 0), stop=(c == 3))
                    Ysb = wpool.tile([P, M], F32, tag="Ysb")
                    nc.scalar.copy(Ysb[:], Yps[:])
                    nc.sync.dma_start(YPOUTv[g], Ysb[:])
                g0 += T
    nc.compile()
    return nc


def _get_nc():
    if "nc" not in _CACHE:
        _CACHE["nc"] = _build_nc()
    return _CACHE["nc"]


def _host_consts():
    iota = np.tile(np.arange(N, dtype=np.float32), (P, 1))
    m1 = np.zeros((P, 896), np.float32)
    m1[np.arange(P), 384 + np.arange(P)] = 1.0
    return iota, m1


def kernel(Y, D, _trace=False):
    from concourse.bass_utils import run_bass_kernel_spmd

    Y = np.ascontiguousarray(Y, dtype=np.float32)
    D = np.ascontiguousarray(D, dtype=np.float32)
    nc = _get_nc()
    iota, m1 = _host_consts()
    in_maps = []
    for c in range(NCORES):
        in_maps.append({
            "Y": Y[c * BLOC:(c + 1) * BLOC],
            "D": D,
            "IOTA": iota,
            "M1": m1,
        })
    res = run_bass_kernel_spmd(nc, in_maps, list(range(NCORES)),
                               trace=_trace)
    _CACHE["last_result"] = res
    Yp = np.concatenate([r["YPOUT"] for r in res.results], axis=0)
    X = np.concatenate([r["XOUT"] for r in res.results], axis=0)
    return (Yp, X)


# revision 25
# speedup vs baseline: 1.0391x; 1.0046x over previous
"""Batch-OMP DictionaryLearning kernel for 8x Trainium2 NeuronCores.

Per core (data-parallel over batch, BLOC=2048 items = 16 tiles of 128):
  G = D @ D.T + eps*I, augmented with its diagonal as a 513th column and
  stored row-major in DRAM for per-item row gathers.
  hbar = Y @ D.T via PE (fp32).
  Incremental Cholesky-OMP (c-form), K=8 iterations:
    argmax |h| via h^2 + InstMax/InstMaxIndex (first-match tie semantics,
    matching jnp.argmax), indirect-DMA gather of the selected G_aug row,
    w_j = c_j[idx] via onehot multiply-reduce, unnormalized
    c'_k = GD - sum_j (w_j*rc_j) c'_j, rank-1 update h -= (yk_k*rc_k) c'_k.
  Batched back-substitution solves L^T x = yk; X assembled on PE via
  diag(x_k) @ onehot_k matmuls; Y_pred = X @ D via PE transposes.

Tiles processed in 3 passes of [6,6,4] (SBUF capacity for c' planes).
"""
import numpy as np

B, M, N, K = 16384, 256, 512, 8
NCORES = 8
BLOC = B // NCORES           # 2048
NT = BLOC // 128             # 16
PASS_T = [6, 6, 4]
TMAX = 6
DIAG_EPS = 1e-5
ROWW = 516                   # padded G_aug row width (513 used)
P = 128
import os as _os
PE_K_MIN = int(_os.environ.get("PE_K_MIN", "3"))
POOL_TILES = int(_os.environ.get("POOL_TILES", "0"))
TINY_GROUPS = int(_os.environ.get("TINY_GROUPS", "6"))
HSQ_DVE = int(_os.environ.get("HSQ_DVE", "0"))
PHG_POOL = int(_os.environ.get("PHG_POOL", "0"))
POOL_JMOD = int(_os.environ.get("POOL_JMOD", "0"))

_CACHE = {}


def _build_nc():
    import concourse.bacc as bacc
    import concourse.mybir as mybir
    from concourse.alu_op_type import AluOpType as aop
    from concourse.bass import IndirectOffsetOnAxis
    from concourse.tile import TileContext

    F32 = mybir.dt.float32
    U32 = mybir.dt.uint32
    AF = mybir.ActivationFunctionType
    AX = mybir.AxisListType.X

    nc = bacc.Bacc(None, target_bir_lowering=False)

    Yd = nc.dram_tensor("Y", [BLOC, M], F32, kind="ExternalInput")
    Dd = nc.dram_tensor("D", [N, M], F32, kind="ExternalInput")
    IOTAd = nc.dram_tensor("IOTA", [P, N], F32, kind="ExternalInput")
    M1d = nc.dram_tensor("M1", [P, 896], F32, kind="ExternalInput")
    XOUT = nc.dram_tensor("XOUT", [BLOC, N], F32, kind="ExternalOutput")
    YPOUT = nc.dram_tensor("YPOUT", [BLOC, M], F32, kind="ExternalOutput")

    with TileContext(nc) as tc:
        with (
            tc.tile_pool(name="dram", bufs=1, space="DRAM") as dpool,
            tc.tile_pool(name="const", bufs=1) as cpool,
            tc.tile_pool(name="mats", bufs=1) as mpool,
            tc.tile_pool(name="state", bufs=1) as spool,
            tc.tile_pool(name="work", bufs=2) as wpool,
            tc.tile_pool(name="ps_t", bufs=2, space="PSUM") as pst,
            tc.tile_pool(name="ps_mm", bufs=4, space="PSUM") as psm,
        ):
            GAUG = dpool.tile([N, ROWW], F32)
            HBARD = dpool.tile([NT, P, N], F32)

            iota = cpool.tile([P, N], F32)
            m1 = cpool.tile([P, 896], F32)
            nc.sync.dma_start(iota[:], IOTAd[:])
            nc.sync.dma_start(m1[:], M1d[:])
            IDM = m1[:, 384:512]                     # [128,128] identity
            IDM_full = cpool.tile([P, P], F32)
            nc.vector.tensor_copy(IDM_full[:], IDM)

            # ---------------- phase A: G_aug + hbar ----------------
            D_sb = mpool.tile([P, 4, M], F32)        # D rows chunked
            nc.sync.dma_start(D_sb[:], Dd.rearrange("(c p) m -> p c m", p=P))
            Dt = mpool.tile([P, 2, N], F32)          # D^T chunked over m
            for nch in range(4):
                for mc in range(2):
                    tp = pst.tile([P, P], F32, tag="tp")
                    nc.tensor.transpose(
                        tp[:], D_sb[:, nch, mc * P:(mc + 1) * P], IDM)
                    nc.scalar.copy(Dt[:, mc, nch * P:(nch + 1) * P], tp[:])

            G_sb = mpool.tile([P, 4, N], F32)
            diagS = mpool.tile([P, 4], F32)
            junkG = wpool.tile([P, N], F32, tag="junkG")
            for c in range(4):
                gp = psm.tile([P, N], F32, tag="mm")
                for mc in range(2):
                    nc.tensor.matmul(
                        gp[:], Dt[:, mc, c * P:(c + 1) * P], Dt[:, mc, :],
                        start=(mc == 0), stop=(mc == 1))
                m1v = m1[:, 384 - c * P: 896 - c * P]
                nc.vector.scalar_tensor_tensor(
                    G_sb[:, c, :], m1v, DIAG_EPS, gp[:], aop.mult, aop.add)
                nc.vector.scalar_tensor_tensor(
                    junkG[:], G_sb[:, c, :], 1.0, m1v, aop.bypass, aop.mult,
                    accum_out=diagS[:, c:c + 1])
            gaug_v = GAUG.rearrange("(c p) w -> p c w", p=P)
            for c in range(4):
                nc.sync.dma_start(gaug_v[:, c, 0:N], G_sb[:, c, :])
                nc.sync.dma_start(gaug_v[:, c, N:N + 1], diagS[:, c:c + 1])

            h_sb = spool.tile([P, TMAX, N], F32)
            Yv = Yd.rearrange("(g p) m -> g p m", p=P)
            for g in range(NT):
                yt = wpool.tile([P, M], F32, tag="yt")
                nc.sync.dma_start(yt[:], Yv[g])
                yT = wpool.tile([P, 2, P], F32, tag="yT")
                for mc in range(2):
                    tp = pst.tile([P, P], F32, tag="tp")
                    nc.tensor.transpose(tp[:], yt[:, mc * P:(mc + 1) * P], IDM)
                    nc.scalar.copy(yT[:, mc, :], tp[:])
                hp = psm.tile([P, N], F32, tag="mm")
                for mc in range(2):
                    nc.tensor.matmul(hp[:], yT[:, mc, :], Dt[:, mc, :],
                                     start=(mc == 0), stop=(mc == 1))
                hb = wpool.tile([P, N], F32, tag="hb")
                nc.scalar.copy(hb[:], hp[:])
                if g < PASS_T[0]:
                    nc.scalar.copy(h_sb[:, g, :], hp[:])
                else:
                    nc.sync.dma_start(HBARD[g], hb[:])

            # ---------------- OMP state ----------------
            cp_sb = spool.tile([P, TMAX, K, N], F32)
            GD = spool.tile([P, TMAX, 2, ROWW], F32)
            hsq = spool.tile([P, 2, N], F32)
            onehot = spool.tile([P, 2, N], F32)
            mx8 = spool.tile([P, TMAX, 8], F32)
            ixu8 = spool.tile([P, NT, K, 8], U32)
            xsave = spool.tile([P, NT, K], F32)
            hsel = spool.tile([P, TMAX], F32)
            idx1f = spool.tile([P, TMAX], F32)
            wraw = spool.tile([P, TMAX, K], F32)
            wnrm = spool.tile([P, TMAX, K], F32)
            wsq3 = spool.tile([P, TMAX, K], F32)
            wsq = spool.tile([P, TMAX], F32)
            wc2 = spool.tile([P, TMAX], F32)
            wc = spool.tile([P, TMAX], F32)
            rcS = spool.tile([P, TMAX, K], F32)
            rcN = spool.tile([P, TMAX, K], F32)
            ykS = spool.tile([P, TMAX, K], F32)
            gS = spool.tile([P, TMAX, K], F32)
            hscal = spool.tile([P, TMAX], F32)
            LS = spool.tile([P, TMAX, K, K], F32)
            junk = spool.tile([P, N], F32)
            junk2 = spool.tile([P, N], F32)
            junk3 = spool.tile([P, N], F32)
            xacc = spool.tile([P, TMAX], F32)
            tmpT = spool.tile([P, TMAX], F32)

            XOUTv = XOUT.rearrange("(g p) n -> g p n", p=P)
            YPOUTv = YPOUT.rearrange("(g p) m -> g p m", p=P)

            g0 = 0
            for pi, T in enumerate(PASS_T):
                for t in range(T):
                    if pi > 0:
                        nc.sync.dma_start(h_sb[:, t, :], HBARD[g0 + t])
                for k in range(K):
                    ngr = max(1, min(TINY_GROUPS, T))
                    szs = [T // ngr + (1 if i < T % ngr else 0)
                           for i in range(ngr)]
                    gb0 = 0
                    for sz in szs:
                        ga, gb = gb0, gb0 + sz
                        gb0 = gb
                        # ---- stage A ----
                        for t in range(ga, gb):
                            g = g0 + t
                            hq = hsq[:, t % 2, :]
                            oh = onehot[:, t % 2, :]
                            if HSQ_DVE:
                                nc.vector.tensor_mul(hq, h_sb[:, t, :],
                                                     h_sb[:, t, :])
                            else:
                                nc.scalar.activation(hq, h_sb[:, t, :],
                                                     AF.Square)
                            nc.vector.max(mx8[:, t, :], hq)
                            nc.vector.max_index(ixu8[:, g, k, :],
                                                mx8[:, t, :], hq)
                            nc.gpsimd.indirect_dma_start(
                                out=GD[:, t, k % 2, :], out_offset=None,
                                in_=GAUG[:, :],
                                in_offset=IndirectOffsetOnAxis(
                                    ap=ixu8[:, g, k, 0:1], axis=0))
                            nc.vector.tensor_copy(idx1f[:, t:t + 1],
                                                  ixu8[:, g, k, 0:1])
                            nc.vector.tensor_scalar(
                                oh, iota[:], idx1f[:, t:t + 1], None,
                                aop.is_equal)
                            seng = nc.gpsimd if t < POOL_TILES else nc.vector
                            seng.scalar_tensor_tensor(
                                junk[:], oh, 1.0, h_sb[:, t, :],
                                aop.bypass, aop.mult,
                                accum_out=hsel[:, t:t + 1])
                            for j in range(k):
                                jeng = (nc.gpsimd if (POOL_JMOD and
                                        j % POOL_JMOD == POOL_JMOD - 1)
                                        else seng)
                                jeng.scalar_tensor_tensor(
                                    junk2[:] if jeng is nc.vector else junk3[:],
                                    cp_sb[:, t, j, :], 1.0, oh,
                                    aop.bypass, aop.mult,
                                    accum_out=wraw[:, t, j:j + 1])
                        # ---- batched tiny algebra (group slice) ----
                        assert gb == ga + 1
                        t0 = ga
                        if k > 0:
                            # wnrm written straight into the L row
                            nc.vector.tensor_mul(
                                LS[:, t0, k, 0:k], wraw[:, ga:gb, 0:k],
                                rcS[:, ga:gb, 0:k])
                            # wsq = sum(wnrm^2) fused via STT self-mult accum
                            nc.vector.scalar_tensor_tensor(
                                wsq3[:, t0, 0:k], LS[:, t0, k, 0:k], 1.0,
                                LS[:, t0, k, 0:k], aop.bypass, aop.mult,
                                accum_out=wsq[:, ga:gb])
                            # wc2n = min(wsq - diag, -eps)  (= -max(diag-wsq, eps))
                            nc.vector.tensor_scalar(
                                wc2[:, ga:gb], wsq[:, ga:gb],
                                GD[:, t0, k % 2, N:N + 1], -DIAG_EPS,
                                aop.subtract, aop.min)
                        else:
                            # wc2n = min(-diag, -eps)
                            nc.vector.tensor_scalar(
                                wc2[:, ga:gb], GD[:, t0, k % 2, N:N + 1], -1.0,
                                -DIAG_EPS, aop.mult, aop.min)
                        nc.scalar.activation(wc[:, ga:gb], wc2[:, ga:gb],
                                             AF.Sqrt, scale=-1.0)
                        nc.vector.reciprocal(rcS[:, ga:gb, k], wc[:, ga:gb])
                        nc.vector.tensor_scalar_mul(
                            rcN[:, ga:gb, k], rcS[:, ga:gb, k], -1.0)
                        nc.vector.tensor_mul(
                            ykS[:, ga:gb, k], hsel[:, ga:gb],
                            rcS[:, ga:gb, k])
                        if k < K - 1:
                            nc.vector.tensor_mul(
                                hscal[:, ga:gb], ykS[:, ga:gb, k],
                                rcN[:, ga:gb, k])
                        if k > 0:
                            nc.vector.tensor_mul(
                                gS[:, ga:gb, 0:k], LS[:, t0, k, 0:k],
                                rcN[:, ga:gb, 0:k])
                        # ---- stage C ----
                        for t in range(ga, gb):
                            if k == K - 1:
                                continue
                            if k == 0:
                                nc.scalar.copy(cp_sb[:, t, 0, :],
                                               GD[:, t, k % 2, 0:N])
                            elif k >= PE_K_MIN:
                                cps = psm.tile([P, N], F32, tag="mm")
                                nc.tensor.matmul(cps[:], IDM_full[:],
                                                 GD[:, t, k % 2, 0:N],
                                                 start=True, stop=False)
                                for j in range(k):
                                    dgj = wpool.tile([P, P], F32, tag="dgj")
                                    nc.scalar.activation(
                                        dgj[:], IDM, AF.Copy,
                                        scale=gS[:, t, j:j + 1])
                                    nc.tensor.matmul(cps[:], dgj[:],
                                                     cp_sb[:, t, j, :],
                                                     start=False,
                                                     stop=(j == k - 1))
                                nc.scalar.copy(cp_sb[:, t, k, :], cps[:])
                            else:
                                nc.vector.scalar_tensor_tensor(
                                    cp_sb[:, t, k, :], cp_sb[:, t, 0, :],
                                    gS[:, t, 0:1], GD[:, t, k % 2, 0:N],
                                    aop.mult, aop.add)
                                for j in range(1, k):
                                    nc.vector.scalar_tensor_tensor(
                                        cp_sb[:, t, k, :], cp_sb[:, t, j, :],
                                        gS[:, t, j:j + 1], cp_sb[:, t, k, :],
                                        aop.mult, aop.add)
                            nc.vector.scalar_tensor_tensor(
                                h_sb[:, t, :], cp_sb[:, t, k, :],
                                hscal[:, t:t + 1], h_sb[:, t, :],
                                aop.mult, aop.add)
                # ---- back-substitution: L^T x = yk ----
                for i in range(K - 1, -1, -1):
                    nc.vector.tensor_copy(xacc[:, 0:T], ykS[:, 0:T, i])
                    for j in range(i + 1, K):
                        nc.vector.tensor_mul(
                            tmpT[:, 0:T], LS[:, 0:T, j, i],
                            xsave[:, g0:g0 + T, j])
                        nc.vector.tensor_sub(
                            xacc[:, 0:T], xacc[:, 0:T], tmpT[:, 0:T])
                    nc.vector.tensor_mul(
                        xsave[:, g0:g0 + T, i], xacc[:, 0:T], rcS[:, 0:T, i])
                # ---- phase G for this pass: X + Y_pred ----
                for t in range(T):
                    g = g0 + t
                    idxf = wpool.tile([P, K], F32, tag="idxf")
                    nc.vector.tensor_copy(idxf[:], ixu8[:, g, :, 0])
                    Xps = psm.tile([P, N], F32, tag="mm")
                    for k in range(K):
                        ohg = wpool.tile([P, N], F32, tag="ohg")
                        oeng = nc.gpsimd if PHG_POOL else nc.vector
                        oeng.tensor_scalar(
                            ohg[:], iota[:], idxf[:, k:k + 1], None,
                            aop.is_equal)
                        dgx = wpool.tile([P, P], F32, tag="dgx")
                        nc.scalar.activation(
                            dgx[:], IDM, AF.Copy, scale=xsave[:, g, k:k + 1])
                        nc.tensor.matmul(Xps[:], dgx[:], ohg[:],
                                         start=(k == 0), stop=(k == K - 1))
                    Xsb = wpool.tile([P, N], F32, tag="Xsb")
                    nc.scalar.copy(Xsb[:], Xps[:])
                    nc.sync.dma_start(XOUTv[g], Xsb[:])
                    Yps = psm.tile([P, M], F32, tag="mm")
                    for c in range(4):
                        tp = pst.tile([P, P], F32, tag="tp")
                        nc.tensor.transpose(
                            tp[:], Xsb[:, c * P:(c + 1) * P], IDM)
                        xT = wpool.tile([P, P], F32, tag="xT")
                        nc.scalar.copy(xT[:], tp[:])
                        nc.tensor.matmul(Yps[:], xT[:], D_sb[:, c, :],
                                         start=(c == 0), stop=(c == 3))
                    Ysb = wpool.tile([P, M], F32, tag="Ysb")
                    nc.scalar.copy(Ysb[:], Yps[:])
                    nc.sync.dma_start(YPOUTv[g], Ysb[:])
                g0 += T
    nc.compile()
    return nc


def _get_nc():
    if "nc" not in _CACHE:
        _CACHE["nc"] = _build_nc()
    return _CACHE["nc"]


def _host_consts():
    iota = np.tile(np.arange(N, dtype=np.float32), (P, 1))
    m1 = np.zeros((P, 896), np.float32)
    m1[np.arange(P), 384 + np.arange(P)] = 1.0
    return iota, m1


def kernel(Y, D, _trace=False):
    from concourse.bass_utils import run_bass_kernel_spmd

    Y = np.ascontiguousarray(Y, dtype=np.float32)
    D = np.ascontiguousarray(D, dtype=np.float32)
    nc = _get_nc()
    iota, m1 = _host_consts()
    in_maps = []
    for c in range(NCORES):
        in_maps.append({
            "Y": Y[c * BLOC:(c + 1) * BLOC],
            "D": D,
            "IOTA": iota,
            "M1": m1,
        })
    res = run_bass_kernel_spmd(nc, in_maps, list(range(NCORES)),
                               trace=_trace)
    _CACHE["last_result"] = res
    Yp = np.concatenate([r["YPOUT"] for r in res.results], axis=0)
    X = np.concatenate([r["XOUT"] for r in res.results], axis=0)
    return (Yp, X)


# revision 28
# speedup vs baseline: 1.0399x; 1.0008x over previous
"""Batch-OMP DictionaryLearning kernel for 8x Trainium2 NeuronCores.

Per core (data-parallel over batch, BLOC=2048 items = 16 tiles of 128):
  G = D @ D.T + eps*I, augmented with its diagonal as a 513th column and
  stored row-major in DRAM for per-item row gathers.
  hbar = Y @ D.T via PE (fp32).
  Incremental Cholesky-OMP (c-form), K=8 iterations:
    argmax |h| via h^2 + InstMax/InstMaxIndex (first-match tie semantics,
    matching jnp.argmax), indirect-DMA gather of the selected G_aug row,
    w_j = c_j[idx] via onehot multiply-reduce, unnormalized
    c'_k = GD - sum_j (w_j*rc_j) c'_j, rank-1 update h -= (yk_k*rc_k) c'_k.
  Batched back-substitution solves L^T x = yk; X assembled on PE via
  diag(x_k) @ onehot_k matmuls; Y_pred = X @ D via PE transposes.

Tiles processed in 3 passes of [6,6,4] (SBUF capacity for c' planes).
"""
import numpy as np

B, M, N, K = 16384, 256, 512, 8
NCORES = 8
BLOC = B // NCORES           # 2048
NT = BLOC // 128             # 16
PASS_T = [6, 6, 4]
TMAX = 6
DIAG_EPS = 1e-5
ROWW = 516                   # padded G_aug row width (513 used)
P = 128
import os as _os
PE_K_MIN = int(_os.environ.get("PE_K_MIN", "3"))
POOL_TILES = int(_os.environ.get("POOL_TILES", "0"))
TINY_GROUPS = int(_os.environ.get("TINY_GROUPS", "6"))
HSQ_DVE = int(_os.environ.get("HSQ_DVE", "0"))
PHG_POOL = int(_os.environ.get("PHG_POOL", "0"))
POOL_JMOD = int(_os.environ.get("POOL_JMOD", "0"))

_CACHE = {}


def _build_nc():
    import concourse.bacc as bacc
    import concourse.mybir as mybir
    from concourse.alu_op_type import AluOpType as aop
    from concourse.bass import IndirectOffsetOnAxis
    from concourse.tile import TileContext

    F32 = mybir.dt.float32
    U32 = mybir.dt.uint32
    AF = mybir.ActivationFunctionType
    AX = mybir.AxisListType.X

    nc = bacc.Bacc(None, target_bir_lowering=False)

    Yd = nc.dram_tensor("Y", [BLOC, M], F32, kind="ExternalInput")
    Dd = nc.dram_tensor("D", [N, M], F32, kind="ExternalInput")
    IOTAd = nc.dram_tensor("IOTA", [P, N], F32, kind="ExternalInput")
    M1d = nc.dram_tensor("M1", [P, 896], F32, kind="ExternalInput")
    XOUT = nc.dram_tensor("XOUT", [BLOC, N], F32, kind="ExternalOutput")
    YPOUT = nc.dram_tensor("YPOUT", [BLOC, M], F32, kind="ExternalOutput")

    with TileContext(nc) as tc:
        with (
            tc.tile_pool(name="dram", bufs=1, space="DRAM") as dpool,
            tc.tile_pool(name="const", bufs=1) as cpool,
            tc.tile_pool(name="mats", bufs=1) as mpool,
            tc.tile_pool(name="state", bufs=1) as spool,
            tc.tile_pool(name="work", bufs=2) as wpool,
            tc.tile_pool(name="ps_t", bufs=4, space="PSUM") as pst,
            tc.tile_pool(name="ps_mm", bufs=4, space="PSUM") as psm,
        ):
            GAUG = dpool.tile([N, ROWW], F32)
            HBARD = dpool.tile([NT, P, N], F32)

            iota = cpool.tile([P, N], F32)
            m1 = cpool.tile([P, 896], F32)
            nc.sync.dma_start(iota[:], IOTAd[:])
            nc.sync.dma_start(m1[:], M1d[:])
            IDM = m1[:, 384:512]                     # [128,128] identity
            IDM_full = cpool.tile([P, P], F32)
            nc.vector.tensor_copy(IDM_full[:], IDM)

            # ---------------- phase A: G_aug + hbar ----------------
            D_sb = mpool.tile([P, 4, M], F32)        # D rows chunked
            nc.sync.dma_start(D_sb[:], Dd.rearrange("(c p) m -> p c m", p=P))
            Dt = mpool.tile([P, 2, N], F32)          # D^T chunked over m
            for nch in range(4):
                for mc in range(2):
                    tp = pst.tile([P, P], F32, tag="tp")
                    nc.tensor.transpose(
                        tp[:], D_sb[:, nch, mc * P:(mc + 1) * P], IDM)
                    nc.scalar.copy(Dt[:, mc, nch * P:(nch + 1) * P], tp[:])

            G_sb = mpool.tile([P, 4, N], F32)
            diagS = mpool.tile([P, 4], F32)
            junkG = wpool.tile([P, N], F32, tag="junkG")
            for c in range(4):
                gp = psm.tile([P, N], F32, tag="mm")
                for mc in range(2):
                    nc.tensor.matmul(
                        gp[:], Dt[:, mc, c * P:(c + 1) * P], Dt[:, mc, :],
                        start=(mc == 0), stop=(mc == 1))
                m1v = m1[:, 384 - c * P: 896 - c * P]
                nc.vector.scalar_tensor_tensor(
                    G_sb[:, c, :], m1v, DIAG_EPS, gp[:], aop.mult, aop.add)
                nc.vector.scalar_tensor_tensor(
                    junkG[:], G_sb[:, c, :], 1.0, m1v, aop.bypass, aop.mult,
                    accum_out=diagS[:, c:c + 1])
            gaug_v = GAUG.rearrange("(c p) w -> p c w", p=P)
            for c in range(4):
                nc.sync.dma_start(gaug_v[:, c, 0:N], G_sb[:, c, :])
                nc.sync.dma_start(gaug_v[:, c, N:N + 1], diagS[:, c:c + 1])

            h_sb = spool.tile([P, TMAX, N], F32)
            Yv = Yd.rearrange("(g p) m -> g p m", p=P)
            for g in range(NT):
                yt = wpool.tile([P, M], F32, tag="yt")
                nc.sync.dma_start(yt[:], Yv[g])
                yT = wpool.tile([P, 2, P], F32, tag="yT")
                for mc in range(2):
                    tp = pst.tile([P, P], F32, tag="tp")
                    nc.tensor.transpose(tp[:], yt[:, mc * P:(mc + 1) * P], IDM)
                    nc.scalar.copy(yT[:, mc, :], tp[:])
                hp = psm.tile([P, N], F32, tag="mm")
                for mc in range(2):
                    nc.tensor.matmul(hp[:], yT[:, mc, :], Dt[:, mc, :],
                                     start=(mc == 0), stop=(mc == 1))
                hb = wpool.tile([P, N], F32, tag="hb")
                nc.scalar.copy(hb[:], hp[:])
                if g < PASS_T[0]:
                    nc.scalar.copy(h_sb[:, g, :], hp[:])
                else:
                    nc.sync.dma_start(HBARD[g], hb[:])

            # ---------------- OMP state ----------------
            cp_sb = spool.tile([P, TMAX, K, N], F32)
            GD = spool.tile([P, TMAX, 2, ROWW], F32)
            hsq = spool.tile([P, 2, N], F32)
            onehot = spool.tile([P, 2, N], F32)
            mx8 = spool.tile([P, TMAX, 8], F32)
            ixu8 = spool.tile([P, NT, K, 8], U32)
            xsave = spool.tile([P, NT, K], F32)
            hsel = spool.tile([P, TMAX], F32)
            idx1f = spool.tile([P, TMAX], F32)
            wraw = spool.tile([P, TMAX, K], F32)
            wnrm = spool.tile([P, TMAX, K], F32)
            wsq3 = spool.tile([P, TMAX, K], F32)
            wsq = spool.tile([P, TMAX], F32)
            wc2 = spool.tile([P, TMAX], F32)
            wc = spool.tile([P, TMAX], F32)
            rcS = spool.tile([P, TMAX, K], F32)
            rcN = spool.tile([P, TMAX, K], F32)
            ykS = spool.tile([P, TMAX, K], F32)
            gS = spool.tile([P, TMAX, K], F32)
            hscal = spool.tile([P, TMAX], F32)
            LS = spool.tile([P, TMAX, K, K], F32)
            junk = spool.tile([P, N], F32)
            junk2 = spool.tile([P, N], F32)
            junk3 = spool.tile([P, N], F32)
            xacc = spool.tile([P, TMAX], F32)
            tmpT = spool.tile([P, TMAX], F32)

            XOUTv = XOUT.rearrange("(g p) n -> g p n", p=P)
            YPOUTv = YPOUT.rearrange("(g p) m -> g p m", p=P)

            g0 = 0
            for pi, T in enumerate(PASS_T):
                for t in range(T):
                    if pi > 0:
                        nc.sync.dma_start(h_sb[:, t, :], HBARD[g0 + t])
                for k in range(K):
                    ngr = max(1, min(TINY_GROUPS, T))
                    szs = [T // ngr + (1 if i < T % ngr else 0)
                           for i in range(ngr)]
                    gb0 = 0
                    for sz in szs:
                        ga, gb = gb0, gb0 + sz
                        gb0 = gb
                        # ---- stage A ----
                        for t in range(ga, gb):
                            g = g0 + t
                            hq = hsq[:, t % 2, :]
                            oh = onehot[:, t % 2, :]
                            if HSQ_DVE:
                                nc.vector.tensor_mul(hq, h_sb[:, t, :],
                                                     h_sb[:, t, :])
                            else:
                                nc.scalar.activation(hq, h_sb[:, t, :],
                                                     AF.Square)
                            nc.vector.max(mx8[:, t, :], hq)
                            nc.vector.max_index(ixu8[:, g, k, :],
                                                mx8[:, t, :], hq)
                            nc.gpsimd.indirect_dma_start(
                                out=GD[:, t, k % 2, :], out_offset=None,
                                in_=GAUG[:, :],
                                in_offset=IndirectOffsetOnAxis(
                                    ap=ixu8[:, g, k, 0:1], axis=0))
                            nc.vector.tensor_copy(idx1f[:, t:t + 1],
                                                  ixu8[:, g, k, 0:1])
                            nc.vector.tensor_scalar(
                                oh, iota[:], idx1f[:, t:t + 1], None,
                                aop.is_equal)
                            seng = nc.gpsimd if t < POOL_TILES else nc.vector
                            seng.scalar_tensor_tensor(
                                junk[:], oh, 1.0, h_sb[:, t, :],
                                aop.bypass, aop.mult,
                                accum_out=hsel[:, t:t + 1])
                            for j in range(k):
                                jeng = (nc.gpsimd if (POOL_JMOD and
                                        j % POOL_JMOD == POOL_JMOD - 1)
                                        else seng)
                                jeng.scalar_tensor_tensor(
                                    junk2[:] if jeng is nc.vector else junk3[:],
                                    cp_sb[:, t, j, :], 1.0, oh,
                                    aop.bypass, aop.mult,
                                    accum_out=wraw[:, t, j:j + 1])
                        # ---- batched tiny algebra (group slice) ----
                        assert gb == ga + 1
                        t0 = ga
                        if k > 0:
                            # wnrm written straight into the L row
                            nc.vector.tensor_mul(
                                LS[:, t0, k, 0:k], wraw[:, ga:gb, 0:k],
                                rcS[:, ga:gb, 0:k])
                            # wsq = sum(wnrm^2) fused via STT self-mult accum
                            nc.vector.scalar_tensor_tensor(
                                wsq3[:, t0, 0:k], LS[:, t0, k, 0:k], 1.0,
                                LS[:, t0, k, 0:k], aop.bypass, aop.mult,
                                accum_out=wsq[:, ga:gb])
                            # wc2n = min(wsq - diag, -eps)  (= -max(diag-wsq, eps))
                            nc.vector.tensor_scalar(
                                wc2[:, ga:gb], wsq[:, ga:gb],
                                GD[:, t0, k % 2, N:N + 1], -DIAG_EPS,
                                aop.subtract, aop.min)
                        else:
                            # wc2n = min(-diag, -eps)
                            nc.vector.tensor_scalar(
                                wc2[:, ga:gb], GD[:, t0, k % 2, N:N + 1], -1.0,
                                -DIAG_EPS, aop.mult, aop.min)
                        nc.scalar.activation(wc[:, ga:gb], wc2[:, ga:gb],
                                             AF.Sqrt, scale=-1.0)
                        nc.vector.reciprocal(rcS[:, ga:gb, k], wc[:, ga:gb])
                        nc.vector.tensor_scalar_mul(
                            rcN[:, ga:gb, k], rcS[:, ga:gb, k], -1.0)
                        nc.vector.tensor_mul(
                            ykS[:, ga:gb, k], hsel[:, ga:gb],
                            rcS[:, ga:gb, k])
                        if k < K - 1:
                            nc.vector.tensor_mul(
                                hscal[:, ga:gb], ykS[:, ga:gb, k],
                                rcN[:, ga:gb, k])
                        if k > 0:
                            nc.vector.tensor_mul(
                                gS[:, ga:gb, 0:k], LS[:, t0, k, 0:k],
                                rcN[:, ga:gb, 0:k])
                        # ---- stage C ----
                        for t in range(ga, gb):
                            if k == K - 1:
                                continue
                            if k == 0:
                                nc.scalar.copy(cp_sb[:, t, 0, :],
                                               GD[:, t, k % 2, 0:N])
                            elif k >= PE_K_MIN:
                                cps = psm.tile([P, N], F32, tag="mm")
                                nc.tensor.matmul(cps[:], IDM_full[:],
                                                 GD[:, t, k % 2, 0:N],
                                                 start=True, stop=False)
                                for j in range(k):
                                    dgj = wpool.tile([P, P], F32, tag="dgj")
                                    nc.scalar.activation(
                                        dgj[:], IDM, AF.Copy,
                                        scale=gS[:, t, j:j + 1])
                                    nc.tensor.matmul(cps[:], dgj[:],
                                                     cp_sb[:, t, j, :],
                                                     start=False,
                                                     stop=(j == k - 1))
                                nc.scalar.copy(cp_sb[:, t, k, :], cps[:])
                            else:
                                nc.vector.scalar_tensor_tensor(
                                    cp_sb[:, t, k, :], cp_sb[:, t, 0, :],
                                    gS[:, t, 0:1], GD[:, t, k % 2, 0:N],
                                    aop.mult, aop.add)
                                for j in range(1, k):
                                    nc.vector.scalar_tensor_tensor(
                                        cp_sb[:, t, k, :], cp_sb[:, t, j, :],
                                        gS[:, t, j:j + 1], cp_sb[:, t, k, :],
                                        aop.mult, aop.add)
                            nc.vector.scalar_tensor_tensor(
                                h_sb[:, t, :], cp_sb[:, t, k, :],
                                hscal[:, t:t + 1], h_sb[:, t, :],
                                aop.mult, aop.add)
                # ---- back-substitution: L^T x = yk ----
                for i in range(K - 1, -1, -1):
                    nc.vector.tensor_copy(xacc[:, 0:T], ykS[:, 0:T, i])
                    for j in range(i + 1, K):
                        nc.vector.tensor_mul(
                            tmpT[:, 0:T], LS[:, 0:T, j, i],
                            xsave[:, g0:g0 + T, j])
                        nc.vector.tensor_sub(
                            xacc[:, 0:T], xacc[:, 0:T], tmpT[:, 0:T])
                    nc.vector.tensor_mul(
                        xsave[:, g0:g0 + T, i], xacc[:, 0:T], rcS[:, 0:T, i])
                # ---- phase G for this pass: X + Y_pred ----
                for t in range(T):
                    g = g0 + t
                    idxf = wpool.tile([P, K], F32, tag="idxf")
                    nc.vector.tensor_copy(idxf[:], ixu8[:, g, :, 0])
                    Xps = psm.tile([P, N], F32, tag="mm")
                    for k in range(K):
                        ohg = wpool.tile([P, N], F32, tag="ohg")
                        oeng = nc.gpsimd if PHG_POOL else nc.vector
                        oeng.tensor_scalar(
                            ohg[:], iota[:], idxf[:, k:k + 1], None,
                            aop.is_equal)
                        dgx = wpool.tile([P, P], F32, tag="dgx")
                        nc.scalar.activation(
                            dgx[:], IDM, AF.Copy, scale=xsave[:, g, k:k + 1])
                        nc.tensor.matmul(Xps[:], dgx[:], ohg[:],
                                         start=(k == 0), stop=(k == K - 1))
                    Xsb = wpool.tile([P, N], F32, tag="Xsb")
                    nc.scalar.copy(Xsb[:], Xps[:])
                    nc.sync.dma_start(XOUTv[g], Xsb[:])
                    Yps = psm.tile([P, M], F32, tag="mm")
                    for c in range(4):
                        tp = pst.tile([P, P], F32, tag="tp")
                        nc.tensor.transpose(
                            tp[:], Xsb[:, c * P:(c + 1) * P], IDM)
                        xT = wpool.tile([P, P], F32, tag="xT")
                        nc.scalar.copy(xT[:], tp[:])
                        nc.tensor.matmul(Yps[:], xT[:], D_sb[:, c, :],
                                         start=(c == 0), stop=(c == 3))
                    Ysb = wpool.tile([P, M], F32, tag="Ysb")
                    nc.scalar.copy(Ysb[:], Yps[:])
                    nc.sync.dma_start(YPOUTv[g], Ysb[:])
                g0 += T
    nc.compile()
    return nc


def _get_nc():
    if "nc" not in _CACHE:
        _CACHE["nc"] = _build_nc()
    return _CACHE["nc"]


def _host_consts():
    iota = np.tile(np.arange(N, dtype=np.float32), (P, 1))
    m1 = np.zeros((P, 896), np.float32)
    m1[np.arange(P), 384 + np.arange(P)] = 1.0
    return iota, m1


def kernel(Y, D, _trace=False):
    from concourse.bass_utils import run_bass_kernel_spmd

    Y = np.ascontiguousarray(Y, dtype=np.float32)
    D = np.ascontiguousarray(D, dtype=np.float32)
    nc = _get_nc()
    iota, m1 = _host_consts()
    in_maps = []
    for c in range(NCORES):
        in_maps.append({
            "Y": Y[c * BLOC:(c + 1) * BLOC],
            "D": D,
            "IOTA": iota,
            "M1": m1,
        })
    res = run_bass_kernel_spmd(nc, in_maps, list(range(NCORES)),
                               trace=_trace)
    _CACHE["last_result"] = res
    Yp = np.concatenate([r["YPOUT"] for r in res.results], axis=0)
    X = np.concatenate([r["XOUT"] for r in res.results], axis=0)
    return (Yp, X)
